# revision 31
# baseline (speedup 1.0000x reference)
"""HSTU block kernel for Trainium2, 8-core data-parallel over batch.

Key layout/scheduling choices:
  - All PE matmul operands are 16-bit (bf16/f16): 1 cycle/row with fp32 PSUM
    accumulation.  x ships as xT [D, N] bf16 (stats + proj rhs) and row-major
    f32 (+b_o folded in) for the residual.
  - The rel-bias ts_w[bucket(log dt)] reconstruction: y = ln|dt| comes from
    two Act ops (Abs with per-partition bias, then Ln -> f16).  Threshold
    indicator tiles t_k = ck*[y >= th_k] are DVE tensor_scalar ops (4x f16
    mode); their SUM is accumulated on the PE via identity matmuls into PSUM
    together with the pos-bias seed, then copied back over acc.  A slice of
    passes runs on Pool (own accumulator) and a slice accumulates on DVE
    (TensorTensor) -- both folded into the same PSUM chain.
  - Causal masking is baked into the bias (-100 on sub-diagonal cells makes
    silu underflow to 0 in f16) -- no affine_select, no qksil memsets; the
    attn@v matmuls restrict columns to the causal region instead.
  - qk logits are produced transposed (LT [key m, query n]); the rel-bias is
    preloaded into PSUM via an f16 identity matmul so the qk matmul
    accumulates on top of it.  Row tiles r<=3 use 1024-wide PSUM tiles (one
    silu per row tile).
  - PSUM budget (8 banks): stats-stack 2 (four [1,512] accumulators live at
    partition offsets 0/32/64/96 of one bank tile), shared [P,512] pool 2
    (proj/qk/av/repl/out), [P,1024] pool 4 (bias chains + wide qk chunks).
"""

import sys

sys.path.insert(0, "/opt/trn_rl_repo")

import numpy as np
import ml_dtypes

import concourse.bass as bass
import concourse.tile as tile
import concourse.mybir as mybir
from concourse import bacc
from concourse.masks import make_identity

B, N, D = 8, 1024, 512
H, DV, DQ = 8, 64, 64
E = 2 * H * DV + 2 * H * DQ  # 2048
EPS = 1e-5
P = 128
NT = N // P  # 8 row tiles
F32 = mybir.dt.float32
F16 = mybir.dt.float16
BF16 = mybir.dt.bfloat16
NPBF = np.dtype(ml_dtypes.bfloat16)
NEG = -100.0  # baked causal-mask bias: silu(x + NEG) == 0 in f16

# threshold-pass distribution knobs
N_POOL_DIAG = 8    # diag passes on Pool (own chain)
N_DVETT_DIAG = 8   # diag passes accumulated on DVE (dacc)
N_DVETT_B1 = 5     # band1 passes accumulated on DVE (dacc2)

_cache = {}


def _bucket(d):
    d = np.maximum(np.abs(d), 1).astype(np.float32)
    return np.clip((np.log(d) / 0.301).astype(np.int32), 0, 128)


def _plan_chunks(ts, tsq):
    """Uniform-across-batch k-ranges for the threshold passes."""
    far = []  # (r, n0, n1, kmin, kmax)
    for r in range(NT):
        n0 = P * (r + 2)
        while n0 < N:
            n1 = min(((n0 // 512) + 1) * 512, N)
            dmin = int((tsq[:, n0] - ts[:, P * r + P - 1]).min())
            dmax = int((tsq[:, n1 - 1] - ts[:, P * r]).max())
            far.append((r, n0, n1, int(_bucket(dmin)), int(_bucket(dmax))))
            n0 = n1
    # diag band: n in [128r, 128r+128), cells n >= m only
    dmin_g = int((tsq - ts).min())
    dmax_g = 0
    for r in range(NT):
        dmax_g = max(dmax_g, int((tsq[:, P * r + P - 1] - ts[:, P * r]).max()))
    kmin_g, kmax_g = int(_bucket(max(dmin_g, 0))), int(_bucket(dmax_g))
    # band1: n in [128(r+1), 128(r+2)) for r=0..6
    d1min = min(int((tsq[:, P * (r + 1)] - ts[:, P * r + P - 1]).min()) for r in range(NT - 1))
    d1max = max(int((tsq[:, P * (r + 2) - 1] - ts[:, P * r]).max()) for r in range(NT - 1))
    k1min, k1max = int(_bucket(max(d1min, 0))), int(_bucket(d1max))
    return far, kmin_g, kmax_g, k1min, k1max


def _build(ts_w_np, far, kmin_g, kmax_g, k1min, k1max):
    nc = bacc.Bacc()
    d = {}
    for name, shape, dt_ in [
        ("xT", [D, N], BF16), ("xr", [N, D], F32), ("tsq_rep", [P, N], F32),
        ("ntsk_col", [P, NT], F32), ("uvqk_g", [D, E], BF16),
        ("bU_col", [P, E // P], F32), ("bUv_row", [1, DV * H], BF16),
        ("W_o", [D, D], BF16),
        ("ga_col", [P, 4], F32), ("bb_col", [P, 4], F32),
        ("vscale_col", [P, NT], F32), ("padout_col", [P, NT], F32),
        ("posacc", [P, 4608], F16),
    ]:
        d[name] = nc.dram_tensor(name, shape, dt_, kind="ExternalInput")
    out_t = nc.dram_tensor("out", [N, D], F32, kind="ExternalOutput")

    widths = [N - P * r for r in range(NT)]
    offs = np.concatenate([[0], np.cumsum(widths)]).astype(int)
    tsw = ts_w_np.astype(np.float64)
    cks = [float(tsw[k] - tsw[k - 1]) for k in range(1, 129)]
    TH = 0.301  # y = ln|d| threshold scale
    AF = mybir.ActivationFunctionType
    OP = mybir.AluOpType

    # pass assignment for the diag band
    kd_all = list(range(kmin_g + 1, kmax_g + 1))
    n_p = min(N_POOL_DIAG, len(kd_all))
    kp_d = kd_all[len(kd_all) - n_p:]
    rest = kd_all[:len(kd_all) - n_p]
    n_t = min(N_DVETT_DIAG, len(rest))
    kt_d = rest[len(rest) - n_t:]
    kpe_d = rest[:len(rest) - n_t]
    kb_all = list(range(k1min + 1, k1max + 1))
    n_tb = min(N_DVETT_B1, len(kb_all))
    kt_b = kb_all[len(kb_all) - n_tb:]
    kpe_b = kb_all[:len(kb_all) - n_tb]

    from contextlib import ExitStack
    with tile.TileContext(nc) as tc, ExitStack() as ctx:
        io = ctx.enter_context(tc.tile_pool(name="io", bufs=1))
        pools = ctx.enter_context(tc.tile_pool(name="work", bufs=4))
        kpool = ctx.enter_context(tc.tile_pool(name="kpool", bufs=6))
        kgpool = ctx.enter_context(tc.tile_pool(name="kgpool", bufs=2))
        pq512 = ctx.enter_context(tc.tile_pool(name="pq512", bufs=2, space="PSUM"))
        pw1024 = ctx.enter_context(tc.tile_pool(name="pw1024", bufs=2, space="PSUM"))
        pstat = ctx.enter_context(tc.tile_pool(name="pstat", bufs=2, space="PSUM"))

        # ---- DMAs (bias-prep inputs first) ----
        tsq_rep = io.tile([P, N], F32, tag="tsqr")
        nc.sync.dma_start(tsq_rep[:], d["tsq_rep"][:])
        small = {}
        for nm, sh, dt_ in [("ntsk_col", [P, NT], F32), ("bU_col", [P, E // P], F32),
                            ("bUv_row", [1, DV * H], BF16),
                            ("ga_col", [P, 4], F32), ("bb_col", [P, 4], F32),
                            ("vscale_col", [P, NT], F32), ("padout_col", [P, NT], F32)]:
            small[nm] = io.tile(sh, dt_, tag=nm, name=nm)
            nc.sync.dma_start(small[nm][:], d[nm][:])
        xT = [io.tile([P, N], BF16, tag=f"xT{s}", name=f"xT{s}") for s in range(4)]
        for s in range(4):
            nc.sync.dma_start(xT[s][:], d["xT"][P * s:P * s + P, :])
        acc = [io.tile([P, widths[r]], F16, tag=f"acc{r}", name=f"acc{r}") for r in range(NT)]
        for r in range(NT):
            nc.sync.dma_start(acc[r][:], d["posacc"][:, offs[r]:offs[r + 1]])

        ident = io.tile([P, P], F16, tag="ident")
        make_identity(nc, ident[:])
        ones_col = io.tile([P, 1], BF16, tag="ones_col")
        nc.vector.memset(ones_col[:], 1.0)
        ones_row = io.tile([1, P], BF16, tag="ones_row")
        nc.vector.memset(ones_row[:], 1.0)

        # ---- rel-bias prep: y = ln|tsq - tsk| per row tile, f16 (Act only) ----
        yh = [io.tile([P, widths[r]], F16, tag=f"yh{r}", name=f"yh{r}") for r in range(NT)]
        ystack = io.tile([P, N], F16, tag="ystack")
        ystack2 = io.tile([P, N - P], F16, tag="ystack2")
        for r in range(NT):
            w = widths[r]
            db = pools.tile([P, N], F32, tag="w32", name="db")
            nc.scalar.activation(db[:, :w], tsq_rep[:, P * r:N], AF.Abs,
                                 bias=small["ntsk_col"][:, r:r + 1], scale=1.0)
            nc.scalar.activation(yh[r][:], db[:, :w], AF.Ln)
            nc.vector.tensor_copy(out=ystack[:, P * r:P * r + P], in_=yh[r][:, 0:P])
            if r < NT - 1:
                nc.vector.tensor_copy(out=ystack2[:, P * r:P * r + P], in_=yh[r][:, P:2 * P])

        # ---- layernorm stats of x: four [1,512] accumulators stacked in one
        # PSUM bank at partition offsets 0/32/64/96 ----
        st1 = pstat.tile([P, 512], F32, tag="st", name="st_x")
        st1b = pstat.tile([P, 512], F32, tag="st", name="st_xb")
        srow = [st1[0:1, :], st1[32:33, :], st1[64:65, :], st1b[0:1, :]]
        for s in range(4):
            sq = pools.tile([P, N], BF16, tag="wb16", name="sq")
            nc.vector.tensor_tensor(sq[:], xT[s][:], xT[s][:], OP.mult)
            for c in range(2):
                nc.tensor.matmul(srow[c][:], ones_col[:],
                                 xT[s][:, 512 * c:512 * c + 512],
                                 start=(s == 0), stop=(s == 3))
                nc.tensor.matmul(srow[2 + c][:], ones_col[:],
                                 sq[:, 512 * c:512 * c + 512],
                                 start=(s == 0), stop=(s == 3))
        mu = io.tile([1, N], BF16, tag="mu")
        rs = io.tile([1, N], BF16, tag="rs")
        tmp1 = pools.tile([1, N], BF16, tag="wsm", name="tmp1")
        for c in range(2):
            nc.vector.tensor_scalar_mul(mu[:, 512 * c:512 * c + 512], srow[c][:], 1.0 / D)
            nc.vector.tensor_scalar_mul(tmp1[:, 512 * c:512 * c + 512], srow[2 + c][:], 1.0 / D)
        mu2 = pools.tile([1, N], BF16, tag="wsm", name="mu2")
        nc.vector.tensor_tensor(mu2[:], mu[:], mu[:], OP.mult)
        nc.vector.tensor_tensor(tmp1[:], tmp1[:], mu2[:], OP.subtract)
        nc.vector.tensor_scalar_add(tmp1[:], tmp1[:], EPS)
        nc.scalar.activation(tmp1[:], tmp1[:], AF.Sqrt)
        with nc.allow_low_precision(reason="bf16 rstd is plenty for 2e-2 tol"):
            nc.vector.reciprocal(rs[:], tmp1[:])

        # replicate mu, rs to [P, N] (bf16)
        mur = io.tile([P, N], BF16, tag="mur")
        rsr = io.tile([P, N], BF16, tag="rsr")
        for vec, rep in [(mu, mur), (rs, rsr)]:
            for c in range(2):
                pt = pq512.tile([P, 512], F32, tag="qk", name="rep")
                nc.tensor.matmul(pt[:], ones_row[:], vec[:, 512 * c:512 * c + 512],
                                 start=True, stop=True)
                nc.scalar.copy(out=rep[:, 512 * c:512 * c + 512], in_=pt[:])

        # xn'T = (xT - mu) * rs  (in place, bf16)
        xnt = xT
        for s in range(4):
            nc.vector.tensor_tensor(xnt[s][:], xT[s][:], mur[:], OP.subtract)
            nc.vector.tensor_tensor(xnt[s][:], xnt[s][:], rsr[:], OP.mult)

        # ---- Pool threshold chain (into gacc) ----
        ystack32 = io.tile([P, N], F32, tag="ys32", name="ystack32")
        nc.gpsimd.tensor_copy(out=ystack32[:], in_=ystack[:])
        gacc = io.tile([P, N], F32, tag="gacc", name="gacc")
        for i, k in enumerate(kp_d):
            if i == 0:
                nc.gpsimd.tensor_scalar(gacc[:], ystack32[:], float(TH * k), cks[k - 1],
                                        OP.is_ge, OP.mult)
            else:
                tg = kgpool.tile([P, N], F32, tag="ktg")
                nc.gpsimd.tensor_scalar(tg[:], ystack32[:], float(TH * k), cks[k - 1],
                                        OP.is_ge, OP.mult)
                nc.gpsimd.tensor_tensor(gacc[:], gacc[:], tg[:], OP.add)
        if not kp_d:
            nc.gpsimd.memset(gacc[:], 0.0)

        # ---- projections interleaved with the PE bias chain ----
        # diag chain accumulates in a [P,1024] PSUM tile: pos seeds + PE-path
        # t_k tiles + dacc + gacc16 folds, then copied back over acc[r][:,0:P].
        pbd = pw1024.tile([P, N], F32, tag="wide", name="bias_diag")

        def diag_seed():
            # start each 512-chunk with the first full-width accumulant
            pass

        # interleave emission: proj tile, then a slice of diag t_k matmuls
        diag_started = [False, False]

        def emit_diag_tk(k):
            t = kpool.tile([P, N], F16, tag="kt")
            nc.vector.tensor_scalar(t[:], ystack[:], float(TH * k), cks[k - 1],
                                    OP.is_ge, OP.mult)
            for c in range(2):
                nc.tensor.matmul(pbd[:, 512 * c:512 * c + 512], ident[:],
                                 t[:, 512 * c:512 * c + 512],
                                 start=(not diag_started[c]), stop=False)
                diag_started[c] = True

        projT = {}
        diag_iter = list(kpe_d)

        def emit_proj_tile(t_idx, dtype):
            projT[t_idx] = io.tile([P, N], dtype, tag=f"pT{t_idx}", name=f"pT{t_idx}")
            uvs = []
            for s in range(4):
                u1 = pools.tile([P, P], BF16, tag="uvs", name="u1")
                nc.sync.dma_start(u1[:], d["uvqk_g"][P * s:P * s + P,
                                                     P * t_idx:P * t_idx + P])
                uvs.append(u1)
            for c in range(2):
                pt = pq512.tile([P, 512], F32, tag="qk", name="proj")
                for s in range(4):
                    nc.tensor.matmul(pt[:], uvs[s][:],
                                     xnt[s][:, 512 * c:512 * c + 512],
                                     start=(s == 0), stop=(s == 3))
                nc.scalar.activation(projT[t_idx][:, 512 * c:512 * c + 512], pt[:],
                                     AF.Silu, bias=small["bU_col"][:, t_idx:t_idx + 1],
                                     scale=1.0)

        for t_idx in range(8, 16):
            emit_proj_tile(t_idx, F16)
            for _ in range(3):
                if diag_iter:
                    emit_diag_tk(diag_iter.pop(0))
        # v row-major, silu + (1-pad)/N scale; bias row folded into the matmul
        vt = [io.tile([P, D], F16, tag=f"v{r}", name=f"v{r}") for r in range(NT)]
        uvv = []
        for s in range(4):
            u2 = pools.tile([P, 512], BF16, tag="uvv", name="u2")
            nc.sync.dma_start(u2[:], d["uvqk_g"][P * s:P * s + P, 512:1024])
            uvv.append(u2)
        for r in range(NT):
            pt = pq512.tile([P, 512], F32, tag="qk", name="projv")
            for s in range(4):
                nc.tensor.matmul(pt[:], xnt[s][:, P * r:P * r + P],
                                 uvv[s][:], start=(s == 0), stop=False)
            nc.tensor.matmul(pt[:], ones_row[:], small["bUv_row"][:],
                             start=False, stop=True)
            tmpv = pools.tile([P, D], F16, tag="wv16", name="tmpv")
            nc.scalar.activation(tmpv[:], pt[:], AF.Silu)
            nc.vector.tensor_scalar(vt[r][:], tmpv[:], small["vscale_col"][:, r:r + 1],
                                    None, OP.mult)
            if diag_iter:
                emit_diag_tk(diag_iter.pop(0))
        while diag_iter:
            emit_diag_tk(diag_iter.pop(0))

        # u projection (consumed only at the final gating multiply, but PE/Act
        # have slack here)
        for t_idx in range(4):
            emit_proj_tile(t_idx, BF16)

        # band1 PE-path indicator tiles + matmuls (emitted before the DVE
        # chains so the DVE queue serves PE first)
        pbb = pw1024.tile([P, N], F32, tag="wide", name="bias_b1")
        b1_started = [False, False]
        for k in kpe_b:
            t = kpool.tile([P, N], F16, tag="kt")
            nc.vector.tensor_scalar(t[:, :N - P], ystack2[:], float(TH * k), cks[k - 1],
                                    OP.is_ge, OP.mult)
            for c in range(2):
                w0, w1 = 512 * c, min(512 * c + 512, N - P)
                nc.tensor.matmul(pbb[:, w0:w1], ident[:], t[:, w0:w1],
                                 start=(not b1_started[c]), stop=False)
                b1_started[c] = True

        # far chunks: per-chunk PSUM accumulation (no DVE chain dependency --
        # finishes early so acc far columns are ready before the heads)
        for (r, n0, n1, kmin, kmax) in far:
            if kmax == kmin:
                continue
            a, b2 = n0 - P * r, n1 - P * r
            w = b2 - a
            pf = pq512.tile([P, 512], F32, tag="qk", name="farc")
            for j, k in enumerate(range(kmin + 1, kmax + 1)):
                t = kpool.tile([P, N], F16, tag="kt")
                nc.vector.tensor_scalar(t[:, :w], yh[r][:, a:b2], float(TH * k),
                                        cks[k - 1], OP.is_ge, OP.mult)
                nc.tensor.matmul(pf[:, :w], ident[:], t[:, :w],
                                 start=(j == 0), stop=False)
            nc.tensor.matmul(pf[:, :w], ident[:], acc[r][:, a:b2],
                             start=False, stop=True)
            nc.scalar.copy(out=acc[r][:, a:b2], in_=pf[:, :w])

        # DVE-accumulated threshold chains (into dacc / dacc2)
        dacc = io.tile([P, N], F16, tag="dacc")
        for i, k in enumerate(kt_d):
            if i == 0:
                nc.vector.tensor_scalar(dacc[:], ystack[:], float(TH * k), cks[k - 1],
                                        OP.is_ge, OP.mult)
            else:
                t = kpool.tile([P, N], F16, tag="kt")
                nc.vector.tensor_scalar(t[:], ystack[:], float(TH * k), cks[k - 1],
                                        OP.is_ge, OP.mult)
                nc.vector.tensor_tensor(dacc[:], dacc[:], t[:], OP.add)
        dacc2 = io.tile([P, N - P], F16, tag="dacc2")
        for i, k in enumerate(kt_b):
            if i == 0:
                nc.vector.tensor_scalar(dacc2[:], ystack2[:], float(TH * k), cks[k - 1],
                                        OP.is_ge, OP.mult)
            else:
                t = kpool.tile([P, N], F16, tag="kt")
                nc.vector.tensor_scalar(t[:, :N - P], ystack2[:], float(TH * k),
                                        cks[k - 1], OP.is_ge, OP.mult)
                nc.vector.tensor_tensor(dacc2[:], dacc2[:], t[:, :N - P], OP.add)

        # fold gacc (via f16 copy) + dacc + pos seeds into the diag chain
        gacc16 = io.tile([P, N], F16, tag="gacc16")
        nc.scalar.copy(out=gacc16[:], in_=gacc[:])
        for c in range(2):
            if kt_d:
                nc.tensor.matmul(pbd[:, 512 * c:512 * c + 512], ident[:],
                                 dacc[:, 512 * c:512 * c + 512],
                                 start=(not diag_started[c]), stop=False)
                diag_started[c] = True
            if kp_d:
                nc.tensor.matmul(pbd[:, 512 * c:512 * c + 512], ident[:],
                                 gacc16[:, 512 * c:512 * c + 512],
                                 start=(not diag_started[c]), stop=False)
                diag_started[c] = True
        for r in range(NT):
            c = r // 4
            nc.tensor.matmul(pbd[:, P * r:P * r + P], ident[:], acc[r][:, 0:P],
                             start=(not diag_started[c]), stop=(r % 4 == 3))
            diag_started[c] = True
        for r in range(NT):
            nc.scalar.copy(out=acc[r][:, 0:P], in_=pbd[:, P * r:P * r + P])

        # band1 finalize: dacc2 fold + pos seeds + copy-out
        if kt_b:
            for c in range(2):
                w0, w1 = 512 * c, min(512 * c + 512, N - P)
                nc.tensor.matmul(pbb[:, w0:w1], ident[:], dacc2[:, w0:w1],
                                 start=(not b1_started[c]), stop=False)
                b1_started[c] = True
        for r in range(NT - 1):
            c = r // 4
            nc.tensor.matmul(pbb[:, P * r:P * r + P], ident[:], acc[r][:, P:2 * P],
                             start=(not b1_started[c]), stop=(r % 4 == 3 or r == NT - 2))
            b1_started[c] = True
        for r in range(NT - 1):
            nc.scalar.copy(out=acc[r][:, P:2 * P], in_=pbb[:, P * r:P * r + P])

        # ---- attention per head ----
        wo = [io.tile([P, D], BF16, tag=f"wo{s}", name=f"wo{s}") for s in range(4)]
        for s in range(4):
            nc.sync.dma_start(wo[s][:], d["W_o"][P * s:P * s + P, :])

        qksil = [io.tile([P, N], F16, tag=f"qs{r}", name=f"qs{r}") for r in range(NT)]
        attnT = [io.tile([P, N], BF16, tag=f"aT{t}", name=f"aT{t}") for t in range(4)]
        st2 = pstat.tile([P, 512], F32, tag="st", name="st_a")
        st2b = pstat.tile([P, 512], F32, tag="st", name="st_ab")
        arow = [st2[0:1, :], st2[32:33, :], st2[64:65, :], st2b[0:1, :]]
        for h in range(H):
            qt = projT[8 + h // 2]
            kt = projT[12 + h // 2]
            pq = 64 * (h % 2)
            for r in range(NT):
                n0 = P * r
                if r < 4:
                    # one wide PSUM tile for the whole row: [n0, 1024)
                    pt = pw1024.tile([P, N], F32, tag="wide", name="qkw")
                    m0 = n0
                    while m0 < N:
                        m1 = min(((m0 // 512) + 1) * 512, N)
                        nc.tensor.matmul(pt[:, m0:m1], ident[:],
                                         acc[r][:, m0 - n0:m1 - n0],
                                         start=True, stop=False)
                        nc.tensor.matmul(pt[:, m0:m1],
                                         kt[pq:pq + 64, P * r:P * r + P],
                                         qt[pq:pq + 64, m0:m1],
                                         start=False, stop=True)
                        m0 = m1
                    nc.scalar.activation(qksil[r][:, n0:N], pt[:, n0:N], AF.Silu)
                else:
                    pt = pq512.tile([P, 512], F32, tag="qk", name="qkn")
                    cw = N - n0
                    nc.tensor.matmul(pt[:, :cw], ident[:], acc[r][:],
                                     start=True, stop=False)
                    nc.tensor.matmul(pt[:, :cw], kt[pq:pq + 64, P * r:P * r + P],
                                     qt[pq:pq + 64, n0:N], start=False, stop=True)
                    nc.scalar.activation(qksil[r][:, n0:N], pt[:, :cw], AF.Silu)
            for c in range(2):
                pa = pq512.tile([P, 512], F32, tag="qk", name="av")
                nsub = min(NT, 4 * (c + 1))
                for r in range(nsub):
                    a = max(0, P * r - 512 * c)
                    nc.tensor.matmul(pa[:64, a:512], vt[r][:, 64 * h:64 * h + 64],
                                     qksil[r][:, 512 * c + a:512 * c + 512],
                                     start=(r == 0), stop=(r == nsub - 1))
                at = attnT[h // 2]
                nc.vector.tensor_copy(out=at[pq:pq + 64, 512 * c:512 * c + 512],
                                      in_=pa[:64, :])
            if h % 2 == 1:
                s = h // 2
                for c in range(2):
                    nc.tensor.matmul(arow[c][:], ones_col[:],
                                     attnT[s][:, 512 * c:512 * c + 512],
                                     start=(s == 0), stop=(s == 3))
                    sqa = pools.tile([P, 512], BF16, tag="wb16", name="sqa")
                    nc.vector.tensor_tensor(sqa[:], attnT[s][:, 512 * c:512 * c + 512],
                                            attnT[s][:, 512 * c:512 * c + 512], OP.mult)
                    nc.tensor.matmul(arow[2 + c][:], ones_col[:], sqa[:],
                                     start=(s == 0), stop=(s == 3))

        # ---- layernorm of attn (stats already accumulated in the heads loop) ----
        mua = io.tile([1, N], BF16, tag="mua")
        rsa = io.tile([1, N], BF16, tag="rsa")
        tmpa = pools.tile([1, N], BF16, tag="wsm", name="tmpa")
        for c in range(2):
            nc.vector.tensor_scalar_mul(mua[:, 512 * c:512 * c + 512], arow[c][:], 1.0 / D)
            nc.vector.tensor_scalar_mul(tmpa[:, 512 * c:512 * c + 512], arow[2 + c][:], 1.0 / D)
        mua2 = pools.tile([1, N], BF16, tag="wsm", name="mua2")
        nc.vector.tensor_tensor(mua2[:], mua[:], mua[:], OP.mult)
        nc.vector.tensor_tensor(tmpa[:], tmpa[:], mua2[:], OP.subtract)
        nc.vector.tensor_scalar_add(tmpa[:], tmpa[:], EPS)
        nc.scalar.activation(tmpa[:], tmpa[:], AF.Sqrt)
        with nc.allow_low_precision(reason="bf16 rstd is plenty for 2e-2 tol"):
            nc.vector.reciprocal(rsa[:], tmpa[:])
        muar = io.tile([P, N], BF16, tag="mur", name="muar")
        rsar = io.tile([P, N], BF16, tag="rsr", name="rsar")
        for vec, rep in [(mua, muar), (rsa, rsar)]:
            for c in range(2):
                pt = pq512.tile([P, 512], F32, tag="qk", name="rep")
                nc.tensor.matmul(pt[:], ones_row[:], vec[:, 512 * c:512 * c + 512],
                                 start=True, stop=True)
                nc.scalar.copy(out=rep[:, 512 * c:512 * c + 512], in_=pt[:])
        # prod = u * (LN_a(attn)*gamma+beta) per column half, then that half's
        # output projection + residual (b_o pre-folded into xr)
        for c in range(2):
            cs = slice(512 * c, 512 * c + 512)
            for s in range(4):
                nc.vector.tensor_tensor(attnT[s][:, cs], attnT[s][:, cs],
                                        muar[:, cs], OP.subtract)
                nc.vector.tensor_tensor(attnT[s][:, cs], attnT[s][:, cs],
                                        rsar[:, cs], OP.mult)
                nc.vector.tensor_scalar(attnT[s][:, cs], attnT[s][:, cs],
                                        small["ga_col"][:, s:s + 1],
                                        small["bb_col"][:, s:s + 1],
                                        OP.mult, OP.add)
                nc.vector.tensor_tensor(attnT[s][:, cs], attnT[s][:, cs],
                                        projT[s][:, cs], OP.mult)
            for t in range(4 * c, 4 * c + 4):
                po = pq512.tile([P, 512], F32, tag="qk", name="outp")
                for s in range(4):
                    nc.tensor.matmul(po[:], attnT[s][:, P * t:P * t + P], wo[s][:],
                                     start=(s == 0), stop=(s == 3))
                xtile = pools.tile([P, D], F32, tag="w32", name="xtile")
                nc.sync.dma_start(xtile[:], d["xr"][P * t:P * t + P, :])
                ot = pools.tile([P, D], F32, tag="w32", name="ot")
                nc.vector.tensor_tensor(ot[:], po[:], xtile[:], OP.add)
                nc.vector.tensor_scalar(ot[:], ot[:], small["padout_col"][:, t:t + 1],
                                        None, OP.mult)
                nc.sync.dma_start(out_t[P * t:P * t + P, :], ot[:])

    nc.compile()
    return nc


def _prep_inputs(inputs):
    x = np.asarray(inputs["x"], dtype=np.float32)
    ts = np.asarray(inputs["timestamps"]).astype(np.int64)
    pad = np.asarray(inputs["pad_mask"]).astype(np.float32)
    uvqk = np.asarray(inputs["uvqk"], dtype=np.float32)
    W_o = np.asarray(inputs["W_o"], dtype=np.float32)
    b_o = np.asarray(inputs["b_o"], dtype=np.float32)
    gx = np.asarray(inputs["gamma_x"], dtype=np.float32)
    bx = np.asarray(inputs["beta_x"], dtype=np.float32)
    ga = np.asarray(inputs["gamma_a"], dtype=np.float32)
    ba = np.asarray(inputs["beta_a"], dtype=np.float32)
    ts_w = np.asarray(inputs["ts_w"], dtype=np.float32)
    pos_w = np.asarray(inputs["pos_w"], dtype=np.float32)

    tsq = np.concatenate([ts[:, 1:], ts[:, -1:]], axis=1)  # [B, N]
    far, kmin_g, kmax_g, k1min, k1max = _plan_chunks(ts, tsq)

    uvqk_g = (uvqk * gx[:, None]).astype(NPBF)
    bU = bx @ uvqk  # [E]
    bU_col = bU.reshape(E // P, P).T.copy()  # [P, E//P]
    bUv_row = bU[512:1024].reshape(1, 512).astype(NPBF)
    ga_col = ga.reshape(4, P).T.copy()
    ba_col = ba.reshape(4, P).T.copy()

    # pos-bias tiles in [m, n] layout + per-chunk base constants
    widths = [N - P * r for r in range(NT)]
    offs = np.concatenate([[0], np.cumsum(widths)]).astype(int)
    posacc = np.zeros((P, int(offs[-1])), np.float32)
    nidx = np.arange(N)
    pidx = np.arange(P)[:, None]
    for r in range(NT):
        m = P * r + pidx
        nn = nidx[None, P * r:]
        posacc[:, offs[r]:offs[r + 1]] = pos_w[nn - m + (N - 1)]
        posacc[:, offs[r]:offs[r] + P] += ts_w[kmin_g]
        if r < NT - 1:
            posacc[:, offs[r] + P:offs[r] + 2 * P] += ts_w[k1min]
        # causal mask baked in: sub-diagonal cells of the diag block get a
        # large negative bias so silu(qk + bias) underflows to 0 in f16
        sub = pidx > nidx[None, :P]
        posacc[:, offs[r]:offs[r] + P] = np.where(
            sub, NEG, posacc[:, offs[r]:offs[r] + P])
    for (r, n0, n1, kmin, kmax) in far:
        posacc[:, offs[r] + n0 - P * r: offs[r] + n1 - P * r] += ts_w[kmin]
    posacc = posacc.astype(np.float16)

    xr = x + b_o[None, None, :]  # residual rows with b_o folded in

    per_core = []
    for b in range(B):
        per_core.append({
            "xT": np.ascontiguousarray(x[b].T).astype(NPBF),
            "xr": np.ascontiguousarray(xr[b]),
            "tsq_rep": np.broadcast_to(tsq[b].astype(np.float32), (P, N)).copy(),
            "ntsk_col": np.ascontiguousarray((-ts[b]).astype(np.float32).reshape(NT, P).T),
            "uvqk_g": uvqk_g, "bU_col": bU_col, "bUv_row": bUv_row,
            "W_o": W_o.astype(NPBF),
            "ga_col": ga_col, "bb_col": ba_col,
            "vscale_col": np.ascontiguousarray(
                ((1.0 - pad[b]) / N).astype(np.float32).reshape(NT, P).T),
            "padout_col": np.ascontiguousarray(
                (1.0 - pad[b]).astype(np.float32).reshape(NT, P).T),
            "posacc": posacc,
        })
    return per_core, (far, kmin_g, kmax_g, k1min, k1max, ts_w)


def kernel(**inputs):
    from concourse.bass_utils import run_bass_kernel_spmd

    per_core, (far, kmin_g, kmax_g, k1min, k1max, ts_w) = _prep_inputs(inputs)
    key = (tuple(far), kmin_g, kmax_g, k1min, k1max, ts_w.tobytes())
    if key not in _cache:
        _cache.clear()
        _cache[key] = _build(ts_w, far, kmin_g, kmax_g, k1min, k1max)
    nc = _cache[key]
    res = run_bass_kernel_spmd(nc, per_core, list(range(B)))
    out = np.stack([res.results[b]["out"] for b in range(B)], axis=0)
    return out.astype(np.float32)


# revision 32
# speedup vs baseline: 1.0209x; 1.0209x over previous
"""HSTU block kernel for Trainium2, 8-core data-parallel over batch.

Key layout/scheduling choices:
  - All PE matmul operands are 16-bit (bf16/f16): 1 cycle/row with fp32 PSUM
    accumulation.  x ships as xT [D, N] bf16 (stats + proj rhs) and row-major
    f32 (+b_o folded in) for the residual.
  - The rel-bias ts_w[bucket(log dt)] reconstruction: y = ln|dt| comes from
    two Act ops (Abs with per-partition bias, then Ln -> f16).  Threshold
    indicator tiles t_k = ck*[y >= th_k] are DVE tensor_scalar ops (4x f16
    mode); their SUM is accumulated on the PE via identity matmuls into PSUM
    together with the pos-bias seed, then copied back over acc.  A slice of
    passes runs on Pool (own accumulator) and a slice accumulates on DVE
    (TensorTensor) -- both folded into the same PSUM chain.
  - Causal masking is baked into the bias (-100 on sub-diagonal cells makes
    silu underflow to 0 in f16) -- no affine_select, no qksil memsets; the
    attn@v matmuls restrict columns to the causal region instead.
  - qk logits are produced transposed (LT [key m, query n]); the rel-bias is
    preloaded into PSUM via an f16 identity matmul so the qk matmul
    accumulates on top of it.  Row tiles r<=3 use 1024-wide PSUM tiles (one
    silu per row tile).
  - PSUM budget (8 banks): stats-stack 2 (four [1,512] accumulators live at
    partition offsets 0/32/64/96 of one bank tile), shared [P,512] pool 2
    (proj/qk/av/repl/out), [P,1024] pool 4 (bias chains + wide qk chunks).
"""

import sys

sys.path.insert(0, "/opt/trn_rl_repo")

import numpy as np
import ml_dtypes

import concourse.bass as bass
import concourse.tile as tile
import concourse.mybir as mybir
from concourse import bacc
from concourse.masks import make_identity

B, N, D = 8, 1024, 512
H, DV, DQ = 8, 64, 64
E = 2 * H * DV + 2 * H * DQ  # 2048
EPS = 1e-5
P = 128
NT = N // P  # 8 row tiles
F32 = mybir.dt.float32
F16 = mybir.dt.float16
BF16 = mybir.dt.bfloat16
NPBF = np.dtype(ml_dtypes.bfloat16)
NEG = -100.0  # baked causal-mask bias: silu(x + NEG) == 0 in f16

# threshold-pass distribution knobs
N_POOL_DIAG = 10   # diag passes on Pool (own chain)
N_DVETT_DIAG = 12  # diag passes accumulated on DVE (dacc)
N_DVETT_B1 = 6     # band1 passes accumulated on DVE (dacc2)

_cache = {}


def _bucket(d):
    d = np.maximum(np.abs(d), 1).astype(np.float32)
    return np.clip((np.log(d) / 0.301).astype(np.int32), 0, 128)


def _plan_chunks(ts, tsq):
    """Uniform-across-batch k-ranges for the threshold passes."""
    far = []  # (r, n0, n1, kmin, kmax)
    for r in range(NT):
        n0 = P * (r + 2)
        while n0 < N:
            n1 = min(((n0 // 512) + 1) * 512, N)
            dmin = int((tsq[:, n0] - ts[:, P * r + P - 1]).min())
            dmax = int((tsq[:, n1 - 1] - ts[:, P * r]).max())
            far.append((r, n0, n1, int(_bucket(dmin)), int(_bucket(dmax))))
            n0 = n1
    # diag band: n in [128r, 128r+128), cells n >= m only
    dmin_g = int((tsq - ts).min())
    dmax_g = 0
    for r in range(NT):
        dmax_g = max(dmax_g, int((tsq[:, P * r + P - 1] - ts[:, P * r]).max()))
    kmin_g, kmax_g = int(_bucket(max(dmin_g, 0))), int(_bucket(dmax_g))
    # band1: n in [128(r+1), 128(r+2)) for r=0..6
    d1min = min(int((tsq[:, P * (r + 1)] - ts[:, P * r + P - 1]).min()) for r in range(NT - 1))
    d1max = max(int((tsq[:, P * (r + 2) - 1] - ts[:, P * r]).max()) for r in range(NT - 1))
    k1min, k1max = int(_bucket(max(d1min, 0))), int(_bucket(d1max))
    return far, kmin_g, kmax_g, k1min, k1max


def _build(ts_w_np, far, kmin_g, kmax_g, k1min, k1max):
    nc = bacc.Bacc()
    d = {}
    for name, shape, dt_ in [
        ("xT", [D, N], BF16), ("xr", [N, D], F32), ("tsq_rep", [P, N], F32),
        ("ntsk_col", [P, NT], F32), ("uvqk_g", [D, E], BF16),
        ("bU_col", [P, E // P], F32), ("bUv_row", [1, DV * H], BF16),
        ("W_o", [D, D], BF16),
        ("ga_col", [P, 4], F32), ("bb_col", [P, 4], F32),
        ("vscale_col", [P, NT], F32), ("padout_col", [P, NT], F32),
        ("posacc", [P, 4608], F16),
    ]:
        d[name] = nc.dram_tensor(name, shape, dt_, kind="ExternalInput")
    out_t = nc.dram_tensor("out", [N, D], F32, kind="ExternalOutput")

    widths = [N - P * r for r in range(NT)]
    offs = np.concatenate([[0], np.cumsum(widths)]).astype(int)
    tsw = ts_w_np.astype(np.float64)
    cks = [float(tsw[k] - tsw[k - 1]) for k in range(1, 129)]
    TH = 0.301  # y = ln|d| threshold scale
    AF = mybir.ActivationFunctionType
    OP = mybir.AluOpType

    # pass assignment for the diag band
    kd_all = list(range(kmin_g + 1, kmax_g + 1))
    n_p = min(N_POOL_DIAG, len(kd_all))
    kp_d = kd_all[len(kd_all) - n_p:]
    rest = kd_all[:len(kd_all) - n_p]
    n_t = min(N_DVETT_DIAG, len(rest))
    kt_d = rest[len(rest) - n_t:]
    kpe_d = rest[:len(rest) - n_t]
    kb_all = list(range(k1min + 1, k1max + 1))
    n_tb = min(N_DVETT_B1, len(kb_all))
    kt_b = kb_all[len(kb_all) - n_tb:]
    kpe_b = kb_all[:len(kb_all) - n_tb]

    from contextlib import ExitStack
    with tile.TileContext(nc) as tc, ExitStack() as ctx:
        io = ctx.enter_context(tc.tile_pool(name="io", bufs=1))
        pools = ctx.enter_context(tc.tile_pool(name="work", bufs=4))
        kpool = ctx.enter_context(tc.tile_pool(name="kpool", bufs=8))
        kgpool = ctx.enter_context(tc.tile_pool(name="kgpool", bufs=2))
        pq512 = ctx.enter_context(tc.tile_pool(name="pq512", bufs=2, space="PSUM"))
        pw1024 = ctx.enter_context(tc.tile_pool(name="pw1024", bufs=2, space="PSUM"))
        pstat = ctx.enter_context(tc.tile_pool(name="pstat", bufs=2, space="PSUM"))

        # ---- DMAs (bias-prep inputs first) ----
        tsq_rep = io.tile([P, N], F32, tag="tsqr")
        nc.sync.dma_start(tsq_rep[:], d["tsq_rep"][:])
        small = {}
        for nm, sh, dt_ in [("ntsk_col", [P, NT], F32), ("bU_col", [P, E // P], F32),
                            ("bUv_row", [1, DV * H], BF16),
                            ("ga_col", [P, 4], F32), ("bb_col", [P, 4], F32),
                            ("vscale_col", [P, NT], F32), ("padout_col", [P, NT], F32)]:
            small[nm] = io.tile(sh, dt_, tag=nm, name=nm)
            nc.sync.dma_start(small[nm][:], d[nm][:])
        xT = [io.tile([P, N], BF16, tag=f"xT{s}", name=f"xT{s}") for s in range(4)]
        for s in range(4):
            nc.sync.dma_start(xT[s][:], d["xT"][P * s:P * s + P, :])
        acc = [io.tile([P, widths[r]], F16, tag=f"acc{r}", name=f"acc{r}") for r in range(NT)]
        for r in range(NT):
            nc.sync.dma_start(acc[r][:], d["posacc"][:, offs[r]:offs[r + 1]])

        ident = io.tile([P, P], F16, tag="ident")
        make_identity(nc, ident[:])
        ones_col = io.tile([P, 1], BF16, tag="ones_col")
        nc.vector.memset(ones_col[:], 1.0)
        ones_row = io.tile([1, P], BF16, tag="ones_row")
        nc.vector.memset(ones_row[:], 1.0)

        # ---- rel-bias prep: y = ln|tsq - tsk| per row tile, f16 (Act only) ----
        yh = [io.tile([P, widths[r]], F16, tag=f"yh{r}", name=f"yh{r}") for r in range(NT)]
        ystack = io.tile([P, N], F16, tag="ystack")
        ystack2 = io.tile([P, N - P], F16, tag="ystack2")
        for r in range(NT):
            w = widths[r]
            db = pools.tile([P, N], F32, tag="w32", name="db")
            nc.scalar.activation(db[:, :w], tsq_rep[:, P * r:N], AF.Abs,
                                 bias=small["ntsk_col"][:, r:r + 1], scale=1.0)
            nc.scalar.activation(yh[r][:], db[:, :w], AF.Ln)
            nc.vector.tensor_copy(out=ystack[:, P * r:P * r + P], in_=yh[r][:, 0:P])
            if r < NT - 1:
                nc.vector.tensor_copy(out=ystack2[:, P * r:P * r + P], in_=yh[r][:, P:2 * P])

        # ---- layernorm stats of x: four [1,512] accumulators stacked in one
        # PSUM bank at partition offsets 0/32/64/96 ----
        st1 = pstat.tile([P, 512], F32, tag="st", name="st_x")
        st1b = pstat.tile([P, 512], F32, tag="st", name="st_xb")
        srow = [st1[0:1, :], st1[32:33, :], st1[64:65, :], st1b[0:1, :]]
        for s in range(4):
            sq = pools.tile([P, N], BF16, tag="wb16", name="sq")
            nc.vector.tensor_tensor(sq[:], xT[s][:], xT[s][:], OP.mult)
            for c in range(2):
                nc.tensor.matmul(srow[c][:], ones_col[:],
                                 xT[s][:, 512 * c:512 * c + 512],
                                 start=(s == 0), stop=(s == 3))
                nc.tensor.matmul(srow[2 + c][:], ones_col[:],
                                 sq[:, 512 * c:512 * c + 512],
                                 start=(s == 0), stop=(s == 3))
        mu = io.tile([1, N], BF16, tag="mu")
        rs = io.tile([1, N], BF16, tag="rs")
        tmp1 = pools.tile([1, N], BF16, tag="wsm", name="tmp1")
        for c in range(2):
            nc.vector.tensor_scalar_mul(mu[:, 512 * c:512 * c + 512], srow[c][:], 1.0 / D)
            nc.vector.tensor_scalar_mul(tmp1[:, 512 * c:512 * c + 512], srow[2 + c][:], 1.0 / D)
        mu2 = pools.tile([1, N], BF16, tag="wsm", name="mu2")
        nc.vector.tensor_tensor(mu2[:], mu[:], mu[:], OP.mult)
        nc.vector.tensor_tensor(tmp1[:], tmp1[:], mu2[:], OP.subtract)
        nc.vector.tensor_scalar_add(tmp1[:], tmp1[:], EPS)
        nc.scalar.activation(tmp1[:], tmp1[:], AF.Sqrt)
        with nc.allow_low_precision(reason="bf16 rstd is plenty for 2e-2 tol"):
            nc.vector.reciprocal(rs[:], tmp1[:])

        # replicate mu, rs to [P, N] (bf16)
        mur = io.tile([P, N], BF16, tag="mur")
        rsr = io.tile([P, N], BF16, tag="rsr")
        for vec, rep in [(mu, mur), (rs, rsr)]:
            for c in range(2):
                pt = pq512.tile([P, 512], F32, tag="qk", name="rep")
                nc.tensor.matmul(pt[:], ones_row[:], vec[:, 512 * c:512 * c + 512],
                                 start=True, stop=True)
                nc.scalar.copy(out=rep[:, 512 * c:512 * c + 512], in_=pt[:])

        # xn'T = (xT - mu) * rs  (in place, bf16)
        xnt = xT
        for s in range(4):
            nc.vector.tensor_tensor(xnt[s][:], xT[s][:], mur[:], OP.subtract)
            nc.vector.tensor_tensor(xnt[s][:], xnt[s][:], rsr[:], OP.mult)

        # ---- DVE-accumulated threshold chains (emitted early: DVE runs them
        # while PE does the projections) ----
        dacc = io.tile([P, N], F16, tag="dacc")
        for i, k in enumerate(kt_d):
            if i == 0:
                nc.vector.tensor_scalar(dacc[:], ystack[:], float(TH * k), cks[k - 1],
                                        OP.is_ge, OP.mult)
            else:
                t = kpool.tile([P, N], F16, tag="kt")
                nc.vector.tensor_scalar(t[:], ystack[:], float(TH * k), cks[k - 1],
                                        OP.is_ge, OP.mult)
                nc.vector.tensor_tensor(dacc[:], dacc[:], t[:], OP.add)
        dacc2 = io.tile([P, N - P], F16, tag="dacc2")
        for i, k in enumerate(kt_b):
            if i == 0:
                nc.vector.tensor_scalar(dacc2[:], ystack2[:], float(TH * k), cks[k - 1],
                                        OP.is_ge, OP.mult)
            else:
                t = kpool.tile([P, N], F16, tag="kt")
                nc.vector.tensor_scalar(t[:, :N - P], ystack2[:], float(TH * k),
                                        cks[k - 1], OP.is_ge, OP.mult)
                nc.vector.tensor_tensor(dacc2[:], dacc2[:], t[:, :N - P], OP.add)

        # ---- Pool threshold chain (into gacc) ----
        ystack32 = io.tile([P, N], F32, tag="ys32", name="ystack32")
        nc.gpsimd.tensor_copy(out=ystack32[:], in_=ystack[:])
        gacc = io.tile([P, N], F32, tag="gacc", name="gacc")
        for i, k in enumerate(kp_d):
            if i == 0:
                nc.gpsimd.tensor_scalar(gacc[:], ystack32[:], float(TH * k), cks[k - 1],
                                        OP.is_ge, OP.mult)
            else:
                tg = kgpool.tile([P, N], F32, tag="ktg")
                nc.gpsimd.tensor_scalar(tg[:], ystack32[:], float(TH * k), cks[k - 1],
                                        OP.is_ge, OP.mult)
                nc.gpsimd.tensor_tensor(gacc[:], gacc[:], tg[:], OP.add)
        if not kp_d:
            nc.gpsimd.memset(gacc[:], 0.0)

        # ---- projections (PE uninterrupted), then the PE bias chains ----
        projT = {}

        def emit_proj_tile(t_idx, dtype):
            projT[t_idx] = io.tile([P, N], dtype, tag=f"pT{t_idx}", name=f"pT{t_idx}")
            uvs = []
            for s in range(4):
                u1 = pools.tile([P, P], BF16, tag="uvs", name="u1")
                nc.sync.dma_start(u1[:], d["uvqk_g"][P * s:P * s + P,
                                                     P * t_idx:P * t_idx + P])
                uvs.append(u1)
            for c in range(2):
                pt = pq512.tile([P, 512], F32, tag="qk", name="proj")
                for s in range(4):
                    nc.tensor.matmul(pt[:], uvs[s][:],
                                     xnt[s][:, 512 * c:512 * c + 512],
                                     start=(s == 0), stop=(s == 3))
                nc.scalar.activation(projT[t_idx][:, 512 * c:512 * c + 512], pt[:],
                                     AF.Silu, bias=small["bU_col"][:, t_idx:t_idx + 1],
                                     scale=1.0)

        for t_idx in range(8, 16):
            emit_proj_tile(t_idx, F16)
        # v row-major, silu + (1-pad)/N scale; bias row folded into the matmul
        vt = [io.tile([P, D], F16, tag=f"v{r}", name=f"v{r}") for r in range(NT)]
        uvv = []
        for s in range(4):
            u2 = pools.tile([P, 512], BF16, tag="uvv", name="u2")
            nc.sync.dma_start(u2[:], d["uvqk_g"][P * s:P * s + P, 512:1024])
            uvv.append(u2)
        for r in range(NT):
            pt = pq512.tile([P, 512], F32, tag="qk", name="projv")
            for s in range(4):
                nc.tensor.matmul(pt[:], xnt[s][:, P * r:P * r + P],
                                 uvv[s][:], start=(s == 0), stop=False)
            nc.tensor.matmul(pt[:], ones_row[:], small["bUv_row"][:],
                             start=False, stop=True)
            tmpv = pools.tile([P, D], F16, tag="wv16", name="tmpv")
            nc.scalar.activation(tmpv[:], pt[:], AF.Silu)
            nc.vector.tensor_scalar(vt[r][:], tmpv[:], small["vscale_col"][:, r:r + 1],
                                    None, OP.mult)
        # u projection (consumed only at the final gating multiply)
        for t_idx in range(4):
            emit_proj_tile(t_idx, BF16)

        # diag PE-path: indicator TSPs (DVE) consumed by identity matmuls
        pbd = pw1024.tile([P, N], F32, tag="wide", name="bias_diag")
        diag_started = [False, False]
        for k in kpe_d:
            t = kpool.tile([P, N], F16, tag="kt")
            nc.vector.tensor_scalar(t[:], ystack[:], float(TH * k), cks[k - 1],
                                    OP.is_ge, OP.mult)
            for c in range(2):
                nc.tensor.matmul(pbd[:, 512 * c:512 * c + 512], ident[:],
                                 t[:, 512 * c:512 * c + 512],
                                 start=(not diag_started[c]), stop=False)
                diag_started[c] = True
        # fold gacc (via f16 copy) + dacc + pos seeds into the diag chain
        gacc16 = io.tile([P, N], F16, tag="gacc16")
        nc.scalar.copy(out=gacc16[:], in_=gacc[:])
        for c in range(2):
            if kt_d:
                nc.tensor.matmul(pbd[:, 512 * c:512 * c + 512], ident[:],
                                 dacc[:, 512 * c:512 * c + 512],
                                 start=(not diag_started[c]), stop=False)
                diag_started[c] = True
            if kp_d:
                nc.tensor.matmul(pbd[:, 512 * c:512 * c + 512], ident[:],
                                 gacc16[:, 512 * c:512 * c + 512],
                                 start=(not diag_started[c]), stop=False)
                diag_started[c] = True
        for r in range(NT):
            c = r // 4
            nc.tensor.matmul(pbd[:, P * r:P * r + P], ident[:], acc[r][:, 0:P],
                             start=(not diag_started[c]), stop=(r % 4 == 3))
            diag_started[c] = True
        for r in range(NT):
            nc.scalar.copy(out=acc[r][:, 0:P], in_=pbd[:, P * r:P * r + P])

        # band1 chain
        pbb = pw1024.tile([P, N], F32, tag="wide", name="bias_b1")
        b1_started = [False, False]
        for k in kpe_b:
            t = kpool.tile([P, N], F16, tag="kt")
            nc.vector.tensor_scalar(t[:, :N - P], ystack2[:], float(TH * k), cks[k - 1],
                                    OP.is_ge, OP.mult)
            for c in range(2):
                w0, w1 = 512 * c, min(512 * c + 512, N - P)
                nc.tensor.matmul(pbb[:, w0:w1], ident[:], t[:, w0:w1],
                                 start=(not b1_started[c]), stop=False)
                b1_started[c] = True
        if kt_b:
            for c in range(2):
                w0, w1 = 512 * c, min(512 * c + 512, N - P)
                nc.tensor.matmul(pbb[:, w0:w1], ident[:], dacc2[:, w0:w1],
                                 start=(not b1_started[c]), stop=False)
                b1_started[c] = True
        for r in range(NT - 1):
            c = r // 4
            nc.tensor.matmul(pbb[:, P * r:P * r + P], ident[:], acc[r][:, P:2 * P],
                             start=(not b1_started[c]), stop=(r % 4 == 3 or r == NT - 2))
            b1_started[c] = True
        for r in range(NT - 1):
            nc.scalar.copy(out=acc[r][:, P:2 * P], in_=pbb[:, P * r:P * r + P])

        # far chunks: per-chunk PSUM accumulation (skip chunks with no passes)
        for (r, n0, n1, kmin, kmax) in far:
            if kmax == kmin:
                continue
            a, b2 = n0 - P * r, n1 - P * r
            w = b2 - a
            pf = pq512.tile([P, 512], F32, tag="qk", name="farc")
            for j, k in enumerate(range(kmin + 1, kmax + 1)):
                t = kpool.tile([P, N], F16, tag="kt")
                nc.vector.tensor_scalar(t[:, :w], yh[r][:, a:b2], float(TH * k),
                                        cks[k - 1], OP.is_ge, OP.mult)
                nc.tensor.matmul(pf[:, :w], ident[:], t[:, :w],
                                 start=(j == 0), stop=False)
            nc.tensor.matmul(pf[:, :w], ident[:], acc[r][:, a:b2],
                             start=False, stop=True)
            nc.scalar.copy(out=acc[r][:, a:b2], in_=pf[:, :w])

        # ---- attention per head ----
        wo = [io.tile([P, D], BF16, tag=f"wo{s}", name=f"wo{s}") for s in range(4)]
        for s in range(4):
            nc.sync.dma_start(wo[s][:], d["W_o"][P * s:P * s + P, :])

        qksil = [io.tile([P, N], F16, tag=f"qs{r}", name=f"qs{r}") for r in range(NT)]
        attnT = [io.tile([P, N], BF16, tag=f"aT{t}", name=f"aT{t}") for t in range(4)]
        st2 = pstat.tile([P, 512], F32, tag="st", name="st_a")
        st2b = pstat.tile([P, 512], F32, tag="st", name="st_ab")
        arow = [st2[0:1, :], st2[32:33, :], st2[64:65, :], st2b[0:1, :]]
        for h in range(H):
            qt = projT[8 + h // 2]
            kt = projT[12 + h // 2]
            pq = 64 * (h % 2)
            for r in range(NT):
                n0 = P * r
                if r < 4:
                    # one wide PSUM tile for the whole row: [n0, 1024)
                    pt = pw1024.tile([P, N], F32, tag="wide", name="qkw")
                    m0 = n0
                    while m0 < N:
                        m1 = min(((m0 // 512) + 1) * 512, N)
                        nc.tensor.matmul(pt[:, m0:m1], ident[:],
                                         acc[r][:, m0 - n0:m1 - n0],
                                         start=True, stop=False)
                        nc.tensor.matmul(pt[:, m0:m1],
                                         kt[pq:pq + 64, P * r:P * r + P],
                                         qt[pq:pq + 64, m0:m1],
                                         start=False, stop=True)
                        m0 = m1
                    nc.scalar.activation(qksil[r][:, n0:N], pt[:, n0:N], AF.Silu)
                else:
                    pt = pq512.tile([P, 512], F32, tag="qk", name="qkn")
                    cw = N - n0
                    nc.tensor.matmul(pt[:, :cw], ident[:], acc[r][:],
                                     start=True, stop=False)
                    nc.tensor.matmul(pt[:, :cw], kt[pq:pq + 64, P * r:P * r + P],
                                     qt[pq:pq + 64, n0:N], start=False, stop=True)
                    nc.scalar.activation(qksil[r][:, n0:N], pt[:, :cw], AF.Silu)
            for c in range(2):
                pa = pq512.tile([P, 512], F32, tag="qk", name="av")
                nsub = min(NT, 4 * (c + 1))
                for r in range(nsub):
                    a = max(0, P * r - 512 * c)
                    nc.tensor.matmul(pa[:64, a:512], vt[r][:, 64 * h:64 * h + 64],
                                     qksil[r][:, 512 * c + a:512 * c + 512],
                                     start=(r == 0), stop=(r == nsub - 1))
                at = attnT[h // 2]
                nc.vector.tensor_copy(out=at[pq:pq + 64, 512 * c:512 * c + 512],
                                      in_=pa[:64, :])
            if h % 2 == 1:
                s = h // 2
                for c in range(2):
                    nc.tensor.matmul(arow[c][:], ones_col[:],
                                     attnT[s][:, 512 * c:512 * c + 512],
                                     start=(s == 0), stop=(s == 3))
                    sqa = pools.tile([P, 512], BF16, tag="wb16", name="sqa")
                    nc.vector.tensor_tensor(sqa[:], attnT[s][:, 512 * c:512 * c + 512],
                                            attnT[s][:, 512 * c:512 * c + 512], OP.mult)
                    nc.tensor.matmul(arow[2 + c][:], ones_col[:], sqa[:],
                                     start=(s == 0), stop=(s == 3))

        # ---- layernorm of attn (stats already accumulated in the heads loop) ----
        mua = io.tile([1, N], BF16, tag="mua")
        rsa = io.tile([1, N], BF16, tag="rsa")
        tmpa = pools.tile([1, N], BF16, tag="wsm", name="tmpa")
        for c in range(2):
            nc.vector.tensor_scalar_mul(mua[:, 512 * c:512 * c + 512], arow[c][:], 1.0 / D)
            nc.vector.tensor_scalar_mul(tmpa[:, 512 * c:512 * c + 512], arow[2 + c][:], 1.0 / D)
        mua2 = pools.tile([1, N], BF16, tag="wsm", name="mua2")
        nc.vector.tensor_tensor(mua2[:], mua[:], mua[:], OP.mult)
        nc.vector.tensor_tensor(tmpa[:], tmpa[:], mua2[:], OP.subtract)
        nc.vector.tensor_scalar_add(tmpa[:], tmpa[:], EPS)
        nc.scalar.activation(tmpa[:], tmpa[:], AF.Sqrt)
        with nc.allow_low_precision(reason="bf16 rstd is plenty for 2e-2 tol"):
            nc.vector.reciprocal(rsa[:], tmpa[:])
        muar = io.tile([P, N], BF16, tag="mur", name="muar")
        rsar = io.tile([P, N], BF16, tag="rsr", name="rsar")
        for vec, rep in [(mua, muar), (rsa, rsar)]:
            for c in range(2):
                pt = pq512.tile([P, 512], F32, tag="qk", name="rep")
                nc.tensor.matmul(pt[:], ones_row[:], vec[:, 512 * c:512 * c + 512],
                                 start=True, stop=True)
                nc.scalar.copy(out=rep[:, 512 * c:512 * c + 512], in_=pt[:])
        # prod = u * (LN_a(attn)*gamma+beta) per column half, then that half's
        # output projection + residual (b_o pre-folded into xr)
        for c in range(2):
            cs = slice(512 * c, 512 * c + 512)
            for s in range(4):
                nc.vector.tensor_tensor(attnT[s][:, cs], attnT[s][:, cs],
                                        muar[:, cs], OP.subtract)
                nc.vector.tensor_tensor(attnT[s][:, cs], attnT[s][:, cs],
                                        rsar[:, cs], OP.mult)
                nc.vector.tensor_scalar(attnT[s][:, cs], attnT[s][:, cs],
                                        small["ga_col"][:, s:s + 1],
                                        small["bb_col"][:, s:s + 1],
                                        OP.mult, OP.add)
                nc.vector.tensor_tensor(attnT[s][:, cs], attnT[s][:, cs],
                                        projT[s][:, cs], OP.mult)
            for t in range(4 * c, 4 * c + 4):
                po = pq512.tile([P, 512], F32, tag="qk", name="outp")
                for s in range(4):
                    nc.tensor.matmul(po[:], attnT[s][:, P * t:P * t + P], wo[s][:],
                                     start=(s == 0), stop=(s == 3))
                xtile = pools.tile([P, D], F32, tag="w32", name="xtile")
                nc.sync.dma_start(xtile[:], d["xr"][P * t:P * t + P, :])
                ot = pools.tile([P, D], F32, tag="w32", name="ot")
                nc.vector.tensor_tensor(ot[:], po[:], xtile[:], OP.add)
                nc.vector.tensor_scalar(ot[:], ot[:], small["padout_col"][:, t:t + 1],
                                        None, OP.mult)
                nc.sync.dma_start(out_t[P * t:P * t + P, :], ot[:])

    nc.compile()
    return nc


def _prep_inputs(inputs):
    x = np.asarray(inputs["x"], dtype=np.float32)
    ts = np.asarray(inputs["timestamps"]).astype(np.int64)
    pad = np.asarray(inputs["pad_mask"]).astype(np.float32)
    uvqk = np.asarray(inputs["uvqk"], dtype=np.float32)
    W_o = np.asarray(inputs["W_o"], dtype=np.float32)
    b_o = np.asarray(inputs["b_o"], dtype=np.float32)
    gx = np.asarray(inputs["gamma_x"], dtype=np.float32)
    bx = np.asarray(inputs["beta_x"], dtype=np.float32)
    ga = np.asarray(inputs["gamma_a"], dtype=np.float32)
    ba = np.asarray(inputs["beta_a"], dtype=np.float32)
    ts_w = np.asarray(inputs["ts_w"], dtype=np.float32)
    pos_w = np.asarray(inputs["pos_w"], dtype=np.float32)

    tsq = np.concatenate([ts[:, 1:], ts[:, -1:]], axis=1)  # [B, N]
    far, kmin_g, kmax_g, k1min, k1max = _plan_chunks(ts, tsq)

    uvqk_g = (uvqk * gx[:, None]).astype(NPBF)
    bU = bx @ uvqk  # [E]
    bU_col = bU.reshape(E // P, P).T.copy()  # [P, E//P]
    bUv_row = bU[512:1024].reshape(1, 512).astype(NPBF)
    ga_col = ga.reshape(4, P).T.copy()
    ba_col = ba.reshape(4, P).T.copy()

    # pos-bias tiles in [m, n] layout + per-chunk base constants
    widths = [N - P * r for r in range(NT)]
    offs = np.concatenate([[0], np.cumsum(widths)]).astype(int)
    posacc = np.zeros((P, int(offs[-1])), np.float32)
    nidx = np.arange(N)
    pidx = np.arange(P)[:, None]
    for r in range(NT):
        m = P * r + pidx
        nn = nidx[None, P * r:]
        posacc[:, offs[r]:offs[r + 1]] = pos_w[nn - m + (N - 1)]
        posacc[:, offs[r]:offs[r] + P] += ts_w[kmin_g]
        if r < NT - 1:
            posacc[:, offs[r] + P:offs[r] + 2 * P] += ts_w[k1min]
        # causal mask baked in: sub-diagonal cells of the diag block get a
        # large negative bias so silu(qk + bias) underflows to 0 in f16
        sub = pidx > nidx[None, :P]
        posacc[:, offs[r]:offs[r] + P] = np.where(
            sub, NEG, posacc[:, offs[r]:offs[r] + P])
    for (r, n0, n1, kmin, kmax) in far:
        posacc[:, offs[r] + n0 - P * r: offs[r] + n1 - P * r] += ts_w[kmin]
    posacc = posacc.astype(np.float16)

    xr = x + b_o[None, None, :]  # residual rows with b_o folded in

    per_core = []
    for b in range(B):
        per_core.append({
            "xT": np.ascontiguousarray(x[b].T).astype(NPBF),
            "xr": np.ascontiguousarray(xr[b]),
            "tsq_rep": np.broadcast_to(tsq[b].astype(np.float32), (P, N)).copy(),
            "ntsk_col": np.ascontiguousarray((-ts[b]).astype(np.float32).reshape(NT, P).T),
            "uvqk_g": uvqk_g, "bU_col": bU_col, "bUv_row": bUv_row,
            "W_o": W_o.astype(NPBF),
            "ga_col": ga_col, "bb_col": ba_col,
            "vscale_col": np.ascontiguousarray(
                ((1.0 - pad[b]) / N).astype(np.float32).reshape(NT, P).T),
            "padout_col": np.ascontiguousarray(
                (1.0 - pad[b]).astype(np.float32).reshape(NT, P).T),
            "posacc": posacc,
        })
    return per_core, (far, kmin_g, kmax_g, k1min, k1max, ts_w)


def kernel(**inputs):
    from concourse.bass_utils import run_bass_kernel_spmd

    per_core, (far, kmin_g, kmax_g, k1min, k1max, ts_w) = _prep_inputs(inputs)
    key = (tuple(far), kmin_g, kmax_g, k1min, k1max, ts_w.tobytes())
    if key not in _cache:
        _cache.clear()
        _cache[key] = _build(ts_w, far, kmin_g, kmax_g, k1min, k1max)
    nc = _cache[key]
    res = run_bass_kernel_spmd(nc, per_core, list(range(B)))
    out = np.stack([res.results[b]["out"] for b in range(B)], axis=0)
    return out.astype(np.float32)


# revision 33
# speedup vs baseline: 1.1233x; 1.1003x over previous
"""HSTU block kernel for Trainium2, 8-core data-parallel over batch.

Key layout/scheduling choices:
  - All PE matmul operands are 16-bit (bf16/f16): 1 cycle/row with fp32 PSUM
    accumulation.  x ships as xT [D, N] bf16 (stats + proj rhs) and row-major
    f32 (+b_o folded in) for the residual.
  - The rel-bias ts_w[bucket(log dt)] reconstruction: y = ln|dt| comes from
    two Act ops (Abs with per-partition bias, then Ln -> f16).  Threshold
    indicator tiles t_k = ck*[y >= th_k] are DVE tensor_scalar ops (4x f16
    mode); their SUM is accumulated on the PE via identity matmuls into PSUM
    together with the pos-bias seed, then copied back over acc.  A slice of
    passes runs on Pool (own accumulator) and a slice accumulates on DVE
    (TensorTensor) -- both folded into the same PSUM chain.
  - Causal masking is baked into the bias (-100 on sub-diagonal cells makes
    silu underflow to 0 in f16) -- no affine_select, no qksil memsets; the
    attn@v matmuls restrict columns to the causal region instead.
  - qk logits are produced transposed (LT [key m, query n]); the rel-bias is
    preloaded into PSUM via an f16 identity matmul so the qk matmul
    accumulates on top of it.  Row tiles r<=3 use 1024-wide PSUM tiles (one
    silu per row tile).
  - PSUM budget (8 banks): stats-stack 2 (four [1,512] accumulators live at
    partition offsets 0/32/64/96 of one bank tile), shared [P,512] pool 2
    (proj/qk/av/repl/out), [P,1024] pool 4 (bias chains + wide qk chunks).
"""

import sys

sys.path.insert(0, "/opt/trn_rl_repo")

import numpy as np
import ml_dtypes

import concourse.bass as bass
import concourse.tile as tile
import concourse.mybir as mybir
from concourse import bacc
from concourse.masks import make_identity

B, N, D = 8, 1024, 512
H, DV, DQ = 8, 64, 64
E = 2 * H * DV + 2 * H * DQ  # 2048
EPS = 1e-5
P = 128
NT = N // P  # 8 row tiles
F32 = mybir.dt.float32
F16 = mybir.dt.float16
BF16 = mybir.dt.bfloat16
NPBF = np.dtype(ml_dtypes.bfloat16)
NEG = -100.0  # baked causal-mask bias: silu(x + NEG) == 0 in f16

# threshold-pass distribution knobs (in units of threshold GROUPS)
PASS_GROUP = 2     # consecutive thresholds merged per pass (bias err <= |ck|)
N_POOL_DIAG = 6    # diag groups on Pool (own chain)

_cache = {}


def _bucket(d):
    d = np.maximum(np.abs(d), 1).astype(np.float32)
    return np.clip((np.log(d) / 0.301).astype(np.int32), 0, 128)


def _plan_chunks(ts, tsq):
    """Uniform-across-batch k-ranges for the threshold passes."""
    far = []  # (r, n0, n1, kmin, kmax)
    for r in range(NT):
        n0 = P * (r + 2)
        while n0 < N:
            n1 = min(((n0 // 512) + 1) * 512, N)
            dmin = int((tsq[:, n0] - ts[:, P * r + P - 1]).min())
            dmax = int((tsq[:, n1 - 1] - ts[:, P * r]).max())
            far.append((r, n0, n1, int(_bucket(dmin)), int(_bucket(dmax))))
            n0 = n1
    # diag band: n in [128r, 128r+128), cells n >= m only
    dmin_g = int((tsq - ts).min())
    dmax_g = 0
    for r in range(NT):
        dmax_g = max(dmax_g, int((tsq[:, P * r + P - 1] - ts[:, P * r]).max()))
    kmin_g, kmax_g = int(_bucket(max(dmin_g, 0))), int(_bucket(dmax_g))
    # band1: n in [128(r+1), 128(r+2)) for r=0..6
    d1min = min(int((tsq[:, P * (r + 1)] - ts[:, P * r + P - 1]).min()) for r in range(NT - 1))
    d1max = max(int((tsq[:, P * (r + 2) - 1] - ts[:, P * r]).max()) for r in range(NT - 1))
    k1min, k1max = int(_bucket(max(d1min, 0))), int(_bucket(d1max))
    return far, kmin_g, kmax_g, k1min, k1max


def _build(ts_w_np, far, kmin_g, kmax_g, k1min, k1max):
    nc = bacc.Bacc()
    d = {}
    for name, shape, dt_ in [
        ("xT", [D, N], BF16), ("xr", [N, D], F32), ("tsq_rep", [P, N], F32),
        ("ntsk_col", [P, NT], F32), ("uvqk_g", [D, E], BF16),
        ("bU_col", [P, E // P], F32), ("bUv_row", [1, DV * H], BF16),
        ("W_o", [D, D], BF16),
        ("ga_col", [P, 4], F32), ("bb_col", [P, 4], F32),
        ("vscale_col", [P, NT], F32), ("padout_col", [P, NT], F32),
        ("posacc", [P, 4608], F16),
    ]:
        d[name] = nc.dram_tensor(name, shape, dt_, kind="ExternalInput")
    out_t = nc.dram_tensor("out", [N, D], F32, kind="ExternalOutput")

    widths = [N - P * r for r in range(NT)]
    offs = np.concatenate([[0], np.cumsum(widths)]).astype(int)
    tsw = ts_w_np.astype(np.float64)
    cks = [float(tsw[k] - tsw[k - 1]) for k in range(1, 129)]
    TH = 0.301  # y = ln|d| threshold scale
    AF = mybir.ActivationFunctionType
    OP = mybir.AluOpType

    def _groups(kmin, kmax, g=PASS_GROUP):
        ks = list(range(kmin + 1, kmax + 1))
        out = []
        i = 0
        while i < len(ks):
            grp = ks[i:i + g]
            out.append((float(TH * grp[0]),
                        float(sum(cks[k - 1] for k in grp))))
            i += g
        return out

    # diag groups: a slice to Pool, the rest to the DVE chain; band1 all DVE;
    # far chunks go through the PE identity-matmul path
    gd_all = _groups(kmin_g, kmax_g)
    n_p = min(N_POOL_DIAG, len(gd_all))
    kp_d = gd_all[len(gd_all) - n_p:]
    kt_d = gd_all[:len(gd_all) - n_p]
    kpe_d = []
    kt_b = _groups(k1min, k1max)
    kpe_b = []

    from contextlib import ExitStack
    with tile.TileContext(nc) as tc, ExitStack() as ctx:
        io = ctx.enter_context(tc.tile_pool(name="io", bufs=1))
        pools = ctx.enter_context(tc.tile_pool(name="work", bufs=4))
        kpool = ctx.enter_context(tc.tile_pool(name="kpool", bufs=8))
        kgpool = ctx.enter_context(tc.tile_pool(name="kgpool", bufs=2))
        pq512 = ctx.enter_context(tc.tile_pool(name="pq512", bufs=2, space="PSUM"))
        pw1024 = ctx.enter_context(tc.tile_pool(name="pw1024", bufs=2, space="PSUM"))
        pstat = ctx.enter_context(tc.tile_pool(name="pstat", bufs=2, space="PSUM"))

        # ---- DMAs (bias-prep inputs first) ----
        tsq_rep = io.tile([P, N], F32, tag="tsqr")
        nc.sync.dma_start(tsq_rep[:], d["tsq_rep"][:])
        small = {}
        for nm, sh, dt_ in [("ntsk_col", [P, NT], F32), ("bU_col", [P, E // P], F32),
                            ("bUv_row", [1, DV * H], BF16),
                            ("ga_col", [P, 4], F32), ("bb_col", [P, 4], F32),
                            ("vscale_col", [P, NT], F32), ("padout_col", [P, NT], F32)]:
            small[nm] = io.tile(sh, dt_, tag=nm, name=nm)
            nc.sync.dma_start(small[nm][:], d[nm][:])
        xT = [io.tile([P, N], BF16, tag=f"xT{s}", name=f"xT{s}") for s in range(4)]
        for s in range(4):
            nc.sync.dma_start(xT[s][:], d["xT"][P * s:P * s + P, :])
        acc = [io.tile([P, widths[r]], F16, tag=f"acc{r}", name=f"acc{r}") for r in range(NT)]
        for r in range(NT):
            nc.sync.dma_start(acc[r][:], d["posacc"][:, offs[r]:offs[r + 1]])

        ident = io.tile([P, P], F16, tag="ident")
        make_identity(nc, ident[:])
        ones_col = io.tile([P, 1], BF16, tag="ones_col")
        nc.vector.memset(ones_col[:], 1.0)
        ones_row = io.tile([1, P], BF16, tag="ones_row")
        nc.vector.memset(ones_row[:], 1.0)

        # ---- rel-bias prep: y = ln|tsq - tsk| per row tile, f16 (Act only) ----
        yh = [io.tile([P, widths[r]], F16, tag=f"yh{r}", name=f"yh{r}") for r in range(NT)]
        ystack = io.tile([P, N], F16, tag="ystack")
        ystack2 = io.tile([P, N - P], F16, tag="ystack2")
        for r in range(NT):
            w = widths[r]
            db = pools.tile([P, N], F32, tag="w32", name="db")
            nc.scalar.activation(db[:, :w], tsq_rep[:, P * r:N], AF.Abs,
                                 bias=small["ntsk_col"][:, r:r + 1], scale=1.0)
            nc.scalar.activation(yh[r][:], db[:, :w], AF.Ln)
            nc.vector.tensor_copy(out=ystack[:, P * r:P * r + P], in_=yh[r][:, 0:P])
            if r < NT - 1:
                nc.vector.tensor_copy(out=ystack2[:, P * r:P * r + P], in_=yh[r][:, P:2 * P])

        # ---- layernorm stats of x: four [1,512] accumulators stacked in one
        # PSUM bank at partition offsets 0/32/64/96 ----
        st1 = pstat.tile([P, 512], F32, tag="st", name="st_x")
        st1b = pstat.tile([P, 512], F32, tag="st", name="st_xb")
        srow = [st1[0:1, :], st1[32:33, :], st1[64:65, :], st1b[0:1, :]]
        for s in range(4):
            sq = pools.tile([P, N], BF16, tag="wb16", name="sq")
            nc.vector.tensor_tensor(sq[:], xT[s][:], xT[s][:], OP.mult)
            for c in range(2):
                nc.tensor.matmul(srow[c][:], ones_col[:],
                                 xT[s][:, 512 * c:512 * c + 512],
                                 start=(s == 0), stop=(s == 3))
                nc.tensor.matmul(srow[2 + c][:], ones_col[:],
                                 sq[:, 512 * c:512 * c + 512],
                                 start=(s == 0), stop=(s == 3))
        mu = io.tile([1, N], BF16, tag="mu")
        rs = io.tile([1, N], BF16, tag="rs")
        tmp1 = pools.tile([1, N], BF16, tag="wsm", name="tmp1")
        for c in range(2):
            nc.vector.tensor_scalar_mul(mu[:, 512 * c:512 * c + 512], srow[c][:], 1.0 / D)
            nc.vector.tensor_scalar_mul(tmp1[:, 512 * c:512 * c + 512], srow[2 + c][:], 1.0 / D)
        mu2 = pools.tile([1, N], BF16, tag="wsm", name="mu2")
        nc.vector.tensor_tensor(mu2[:], mu[:], mu[:], OP.mult)
        nc.vector.tensor_tensor(tmp1[:], tmp1[:], mu2[:], OP.subtract)
        nc.vector.tensor_scalar_add(tmp1[:], tmp1[:], EPS)
        nc.scalar.activation(tmp1[:], tmp1[:], AF.Sqrt)
        with nc.allow_low_precision(reason="bf16 rstd is plenty for 2e-2 tol"):
            nc.vector.reciprocal(rs[:], tmp1[:])

        # replicate mu, rs to [P, N] (bf16)
        mur = io.tile([P, N], BF16, tag="mur")
        rsr = io.tile([P, N], BF16, tag="rsr")
        for vec, rep in [(mu, mur), (rs, rsr)]:
            for c in range(2):
                pt = pq512.tile([P, 512], F32, tag="qk", name="rep")
                nc.tensor.matmul(pt[:], ones_row[:], vec[:, 512 * c:512 * c + 512],
                                 start=True, stop=True)
                nc.scalar.copy(out=rep[:, 512 * c:512 * c + 512], in_=pt[:])

        # xn'T = (xT - mu) * rs  (in place, bf16)
        xnt = xT
        for s in range(4):
            nc.vector.tensor_tensor(xnt[s][:], xT[s][:], mur[:], OP.subtract)
            nc.vector.tensor_tensor(xnt[s][:], xnt[s][:], rsr[:], OP.mult)

        # ---- DVE-accumulated threshold chains (emitted early: DVE runs them
        # while PE does the projections) ----
        dacc = io.tile([P, N], F16, tag="dacc")
        for i, (th, cf) in enumerate(kt_d):
            if i == 0:
                nc.vector.tensor_scalar(dacc[:], ystack[:], th, cf,
                                        OP.is_ge, OP.mult)
            else:
                t = kpool.tile([P, N], F16, tag="kt")
                nc.vector.tensor_scalar(t[:], ystack[:], th, cf,
                                        OP.is_ge, OP.mult)
                nc.vector.tensor_tensor(dacc[:], dacc[:], t[:], OP.add)
        dacc2 = io.tile([P, N - P], F16, tag="dacc2")
        for i, (th, cf) in enumerate(kt_b):
            if i == 0:
                nc.vector.tensor_scalar(dacc2[:], ystack2[:], th, cf,
                                        OP.is_ge, OP.mult)
            else:
                t = kpool.tile([P, N], F16, tag="kt")
                nc.vector.tensor_scalar(t[:, :N - P], ystack2[:], th, cf,
                                        OP.is_ge, OP.mult)
                nc.vector.tensor_tensor(dacc2[:], dacc2[:], t[:, :N - P], OP.add)

        # ---- Pool threshold chain (into gacc) ----
        ystack32 = io.tile([P, N], F32, tag="ys32", name="ystack32")
        nc.gpsimd.tensor_copy(out=ystack32[:], in_=ystack[:])
        gacc = io.tile([P, N], F32, tag="gacc", name="gacc")
        for i, (th, cf) in enumerate(kp_d):
            if i == 0:
                nc.gpsimd.tensor_scalar(gacc[:], ystack32[:], th, cf,
                                        OP.is_ge, OP.mult)
            else:
                tg = kgpool.tile([P, N], F32, tag="ktg")
                nc.gpsimd.tensor_scalar(tg[:], ystack32[:], th, cf,
                                        OP.is_ge, OP.mult)
                nc.gpsimd.tensor_tensor(gacc[:], gacc[:], tg[:], OP.add)
        if not kp_d:
            nc.gpsimd.memset(gacc[:], 0.0)

        # ---- projections (PE uninterrupted), then the PE bias chains ----
        projT = {}

        def emit_proj_tile(t_idx, dtype):
            projT[t_idx] = io.tile([P, N], dtype, tag=f"pT{t_idx}", name=f"pT{t_idx}")
            uvs = []
            for s in range(4):
                u1 = pools.tile([P, P], BF16, tag="uvs", name="u1")
                nc.sync.dma_start(u1[:], d["uvqk_g"][P * s:P * s + P,
                                                     P * t_idx:P * t_idx + P])
                uvs.append(u1)
            for c in range(2):
                pt = pq512.tile([P, 512], F32, tag="qk", name="proj")
                for s in range(4):
                    nc.tensor.matmul(pt[:], uvs[s][:],
                                     xnt[s][:, 512 * c:512 * c + 512],
                                     start=(s == 0), stop=(s == 3))
                nc.scalar.activation(projT[t_idx][:, 512 * c:512 * c + 512], pt[:],
                                     AF.Silu, bias=small["bU_col"][:, t_idx:t_idx + 1],
                                     scale=1.0)

        for t_idx in range(8, 16):
            emit_proj_tile(t_idx, F16)
        # v row-major, silu + (1-pad)/N scale; bias row folded into the matmul
        vt = [io.tile([P, D], F16, tag=f"v{r}", name=f"v{r}") for r in range(NT)]
        uvv = []
        for s in range(4):
            u2 = pools.tile([P, 512], BF16, tag="uvv", name="u2")
            nc.sync.dma_start(u2[:], d["uvqk_g"][P * s:P * s + P, 512:1024])
            uvv.append(u2)
        for r in range(NT):
            pt = pq512.tile([P, 512], F32, tag="qk", name="projv")
            for s in range(4):
                nc.tensor.matmul(pt[:], xnt[s][:, P * r:P * r + P],
                                 uvv[s][:], start=(s == 0), stop=False)
            nc.tensor.matmul(pt[:], ones_row[:], small["bUv_row"][:],
                             start=False, stop=True)
            tmpv = pools.tile([P, D], F16, tag="wv16", name="tmpv")
            nc.scalar.activation(tmpv[:], pt[:], AF.Silu)
            nc.vector.tensor_scalar(vt[r][:], tmpv[:], small["vscale_col"][:, r:r + 1],
                                    None, OP.mult)
        # u projection (consumed only at the final gating multiply)
        for t_idx in range(4):
            emit_proj_tile(t_idx, BF16)

        # diag PE-path: indicator TSPs (DVE) consumed by identity matmuls
        pbd = pw1024.tile([P, N], F32, tag="wide", name="bias_diag")
        diag_started = [False, False]
        for (th, cf) in kpe_d:
            t = kpool.tile([P, N], F16, tag="kt")
            nc.vector.tensor_scalar(t[:], ystack[:], th, cf, OP.is_ge, OP.mult)
            for c in range(2):
                nc.tensor.matmul(pbd[:, 512 * c:512 * c + 512], ident[:],
                                 t[:, 512 * c:512 * c + 512],
                                 start=(not diag_started[c]), stop=False)
                diag_started[c] = True
        # fold gacc (via f16 copy) + dacc + pos seeds into the diag chain
        gacc16 = io.tile([P, N], F16, tag="gacc16")
        nc.scalar.copy(out=gacc16[:], in_=gacc[:])
        for c in range(2):
            if kt_d:
                nc.tensor.matmul(pbd[:, 512 * c:512 * c + 512], ident[:],
                                 dacc[:, 512 * c:512 * c + 512],
                                 start=(not diag_started[c]), stop=False)
                diag_started[c] = True
            if kp_d:
                nc.tensor.matmul(pbd[:, 512 * c:512 * c + 512], ident[:],
                                 gacc16[:, 512 * c:512 * c + 512],
                                 start=(not diag_started[c]), stop=False)
                diag_started[c] = True
        for r in range(NT):
            c = r // 4
            nc.tensor.matmul(pbd[:, P * r:P * r + P], ident[:], acc[r][:, 0:P],
                             start=(not diag_started[c]), stop=(r % 4 == 3))
            diag_started[c] = True
        for r in range(NT):
            nc.scalar.copy(out=acc[r][:, 0:P], in_=pbd[:, P * r:P * r + P])

        # band1 chain
        pbb = pw1024.tile([P, N], F32, tag="wide", name="bias_b1")
        b1_started = [False, False]
        for (th, cf) in kpe_b:
            t = kpool.tile([P, N], F16, tag="kt")
            nc.vector.tensor_scalar(t[:, :N - P], ystack2[:], th, cf,
                                    OP.is_ge, OP.mult)
            for c in range(2):
                w0, w1 = 512 * c, min(512 * c + 512, N - P)
                nc.tensor.matmul(pbb[:, w0:w1], ident[:], t[:, w0:w1],
                                 start=(not b1_started[c]), stop=False)
                b1_started[c] = True
        if kt_b:
            for c in range(2):
                w0, w1 = 512 * c, min(512 * c + 512, N - P)
                nc.tensor.matmul(pbb[:, w0:w1], ident[:], dacc2[:, w0:w1],
                                 start=(not b1_started[c]), stop=False)
                b1_started[c] = True
        for r in range(NT - 1):
            c = r // 4
            nc.tensor.matmul(pbb[:, P * r:P * r + P], ident[:], acc[r][:, P:2 * P],
                             start=(not b1_started[c]), stop=(r % 4 == 3 or r == NT - 2))
            b1_started[c] = True
        for r in range(NT - 1):
            nc.scalar.copy(out=acc[r][:, P:2 * P], in_=pbb[:, P * r:P * r + P])

        # far chunks: per-chunk PSUM accumulation (skip chunks with no passes)
        for (r, n0, n1, kmin, kmax) in far:
            if kmax == kmin:
                continue
            a, b2 = n0 - P * r, n1 - P * r
            w = b2 - a
            pf = pq512.tile([P, 512], F32, tag="qk", name="farc")
            for j, (th, cf) in enumerate(_groups(kmin, kmax)):
                t = kpool.tile([P, N], F16, tag="kt")
                nc.vector.tensor_scalar(t[:, :w], yh[r][:, a:b2], th, cf,
                                        OP.is_ge, OP.mult)
                nc.tensor.matmul(pf[:, :w], ident[:], t[:, :w],
                                 start=(j == 0), stop=False)
            nc.tensor.matmul(pf[:, :w], ident[:], acc[r][:, a:b2],
                             start=False, stop=True)
            nc.scalar.copy(out=acc[r][:, a:b2], in_=pf[:, :w])

        # ---- attention per head ----
        wo = [io.tile([P, D], BF16, tag=f"wo{s}", name=f"wo{s}") for s in range(4)]
        for s in range(4):
            nc.sync.dma_start(wo[s][:], d["W_o"][P * s:P * s + P, :])

        qksil = [io.tile([P, N], F16, tag=f"qs{r}", name=f"qs{r}") for r in range(NT)]
        attnT = [io.tile([P, N], BF16, tag=f"aT{t}", name=f"aT{t}") for t in range(4)]
        st2 = pstat.tile([P, 512], F32, tag="st", name="st_a")
        st2b = pstat.tile([P, 512], F32, tag="st", name="st_ab")
        arow = [st2[0:1, :], st2[32:33, :], st2[64:65, :], st2b[0:1, :]]
        for h in range(H):
            qt = projT[8 + h // 2]
            kt = projT[12 + h // 2]
            pq = 64 * (h % 2)
            for r in range(NT):
                n0 = P * r
                if r < 4:
                    # one wide PSUM tile for the whole row: [n0, 1024)
                    pt = pw1024.tile([P, N], F32, tag="wide", name="qkw")
                    m0 = n0
                    while m0 < N:
                        m1 = min(((m0 // 512) + 1) * 512, N)
                        nc.tensor.matmul(pt[:, m0:m1], ident[:],
                                         acc[r][:, m0 - n0:m1 - n0],
                                         start=True, stop=False)
                        nc.tensor.matmul(pt[:, m0:m1],
                                         kt[pq:pq + 64, P * r:P * r + P],
                                         qt[pq:pq + 64, m0:m1],
                                         start=False, stop=True)
                        m0 = m1
                    nc.scalar.activation(qksil[r][:, n0:N], pt[:, n0:N], AF.Silu)
                else:
                    pt = pq512.tile([P, 512], F32, tag="qk", name="qkn")
                    cw = N - n0
                    nc.tensor.matmul(pt[:, :cw], ident[:], acc[r][:],
                                     start=True, stop=False)
                    nc.tensor.matmul(pt[:, :cw], kt[pq:pq + 64, P * r:P * r + P],
                                     qt[pq:pq + 64, n0:N], start=False, stop=True)
                    nc.scalar.activation(qksil[r][:, n0:N], pt[:, :cw], AF.Silu)
            for c in range(2):
                pa = pq512.tile([P, 512], F32, tag="qk", name="av")
                nsub = min(NT, 4 * (c + 1))
                for r in range(nsub):
                    a = max(0, P * r - 512 * c)
                    nc.tensor.matmul(pa[:64, a:512], vt[r][:, 64 * h:64 * h + 64],
                                     qksil[r][:, 512 * c + a:512 * c + 512],
                                     start=(r == 0), stop=(r == nsub - 1))
                at = attnT[h // 2]
                nc.vector.tensor_copy(out=at[pq:pq + 64, 512 * c:512 * c + 512],
                                      in_=pa[:64, :])
            if h % 2 == 1:
                s = h // 2
                for c in range(2):
                    nc.tensor.matmul(arow[c][:], ones_col[:],
                                     attnT[s][:, 512 * c:512 * c + 512],
                                     start=(s == 0), stop=(s == 3))
                    sqa = pools.tile([P, 512], BF16, tag="wb16", name="sqa")
                    nc.vector.tensor_tensor(sqa[:], attnT[s][:, 512 * c:512 * c + 512],
                                            attnT[s][:, 512 * c:512 * c + 512], OP.mult)
                    nc.tensor.matmul(arow[2 + c][:], ones_col[:], sqa[:],
                                     start=(s == 0), stop=(s == 3))

        # ---- layernorm of attn (stats already accumulated in the heads loop) ----
        mua = io.tile([1, N], BF16, tag="mua")
        rsa = io.tile([1, N], BF16, tag="rsa")
        tmpa = pools.tile([1, N], BF16, tag="wsm", name="tmpa")
        for c in range(2):
            nc.vector.tensor_scalar_mul(mua[:, 512 * c:512 * c + 512], arow[c][:], 1.0 / D)
            nc.vector.tensor_scalar_mul(tmpa[:, 512 * c:512 * c + 512], arow[2 + c][:], 1.0 / D)
        mua2 = pools.tile([1, N], BF16, tag="wsm", name="mua2")
        nc.vector.tensor_tensor(mua2[:], mua[:], mua[:], OP.mult)
        nc.vector.tensor_tensor(tmpa[:], tmpa[:], mua2[:], OP.subtract)
        nc.vector.tensor_scalar_add(tmpa[:], tmpa[:], EPS)
        nc.scalar.activation(tmpa[:], tmpa[:], AF.Sqrt)
        with nc.allow_low_precision(reason="bf16 rstd is plenty for 2e-2 tol"):
            nc.vector.reciprocal(rsa[:], tmpa[:])
        muar = io.tile([P, N], BF16, tag="mur", name="muar")
        rsar = io.tile([P, N], BF16, tag="rsr", name="rsar")
        for vec, rep in [(mua, muar), (rsa, rsar)]:
            for c in range(2):
                pt = pq512.tile([P, 512], F32, tag="qk", name="rep")
                nc.tensor.matmul(pt[:], ones_row[:], vec[:, 512 * c:512 * c + 512],
                                 start=True, stop=True)
                nc.scalar.copy(out=rep[:, 512 * c:512 * c + 512], in_=pt[:])
        # prod = u * (LN_a(attn)*gamma+beta) per column half, then that half's
        # output projection + residual (b_o pre-folded into xr)
        for c in range(2):
            cs = slice(512 * c, 512 * c + 512)
            for s in range(4):
                nc.vector.tensor_tensor(attnT[s][:, cs], attnT[s][:, cs],
                                        muar[:, cs], OP.subtract)
                nc.vector.tensor_tensor(attnT[s][:, cs], attnT[s][:, cs],
                                        rsar[:, cs], OP.mult)
                nc.vector.tensor_scalar(attnT[s][:, cs], attnT[s][:, cs],
                                        small["ga_col"][:, s:s + 1],
                                        small["bb_col"][:, s:s + 1],
                                        OP.mult, OP.add)
                nc.vector.tensor_tensor(attnT[s][:, cs], attnT[s][:, cs],
                                        projT[s][:, cs], OP.mult)
            for t in range(4 * c, 4 * c + 4):
                po = pq512.tile([P, 512], F32, tag="qk", name="outp")
                for s in range(4):
                    nc.tensor.matmul(po[:], attnT[s][:, P * t:P * t + P], wo[s][:],
                                     start=(s == 0), stop=(s == 3))
                xtile = pools.tile([P, D], F32, tag="w32", name="xtile")
                nc.sync.dma_start(xtile[:], d["xr"][P * t:P * t + P, :])
                ot = pools.tile([P, D], F32, tag="w32", name="ot")
                nc.vector.tensor_tensor(ot[:], po[:], xtile[:], OP.add)
                nc.vector.tensor_scalar(ot[:], ot[:], small["padout_col"][:, t:t + 1],
                                        None, OP.mult)
                nc.sync.dma_start(out_t[P * t:P * t + P, :], ot[:])

    nc.compile()
    return nc


def _prep_inputs(inputs):
    x = np.asarray(inputs["x"], dtype=np.float32)
    ts = np.asarray(inputs["timestamps"]).astype(np.int64)
    pad = np.asarray(inputs["pad_mask"]).astype(np.float32)
    uvqk = np.asarray(inputs["uvqk"], dtype=np.float32)
    W_o = np.asarray(inputs["W_o"], dtype=np.float32)
    b_o = np.asarray(inputs["b_o"], dtype=np.float32)
    gx = np.asarray(inputs["gamma_x"], dtype=np.float32)
    bx = np.asarray(inputs["beta_x"], dtype=np.float32)
    ga = np.asarray(inputs["gamma_a"], dtype=np.float32)
    ba = np.asarray(inputs["beta_a"], dtype=np.float32)
    ts_w = np.asarray(inputs["ts_w"], dtype=np.float32)
    pos_w = np.asarray(inputs["pos_w"], dtype=np.float32)

    tsq = np.concatenate([ts[:, 1:], ts[:, -1:]], axis=1)  # [B, N]
    far, kmin_g, kmax_g, k1min, k1max = _plan_chunks(ts, tsq)

    uvqk_g = (uvqk * gx[:, None]).astype(NPBF)
    bU = bx @ uvqk  # [E]
    bU_col = bU.reshape(E // P, P).T.copy()  # [P, E//P]
    bUv_row = bU[512:1024].reshape(1, 512).astype(NPBF)
    ga_col = ga.reshape(4, P).T.copy()
    ba_col = ba.reshape(4, P).T.copy()

    # pos-bias tiles in [m, n] layout + per-chunk base constants
    widths = [N - P * r for r in range(NT)]
    offs = np.concatenate([[0], np.cumsum(widths)]).astype(int)
    posacc = np.zeros((P, int(offs[-1])), np.float32)
    nidx = np.arange(N)
    pidx = np.arange(P)[:, None]
    for r in range(NT):
        m = P * r + pidx
        nn = nidx[None, P * r:]
        posacc[:, offs[r]:offs[r + 1]] = pos_w[nn - m + (N - 1)]
        posacc[:, offs[r]:offs[r] + P] += ts_w[kmin_g]
        if r < NT - 1:
            posacc[:, offs[r] + P:offs[r] + 2 * P] += ts_w[k1min]
        # causal mask baked in: sub-diagonal cells of the diag block get a
        # large negative bias so silu(qk + bias) underflows to 0 in f16
        sub = pidx > nidx[None, :P]
        posacc[:, offs[r]:offs[r] + P] = np.where(
            sub, NEG, posacc[:, offs[r]:offs[r] + P])
    for (r, n0, n1, kmin, kmax) in far:
        posacc[:, offs[r] + n0 - P * r: offs[r] + n1 - P * r] += ts_w[kmin]
    posacc = posacc.astype(np.float16)

    xr = x + b_o[None, None, :]  # residual rows with b_o folded in

    per_core = []
    for b in range(B):
        per_core.append({
            "xT": np.ascontiguousarray(x[b].T).astype(NPBF),
            "xr": np.ascontiguousarray(xr[b]),
            "tsq_rep": np.broadcast_to(tsq[b].astype(np.float32), (P, N)).copy(),
            "ntsk_col": np.ascontiguousarray((-ts[b]).astype(np.float32).reshape(NT, P).T),
            "uvqk_g": uvqk_g, "bU_col": bU_col, "bUv_row": bUv_row,
            "W_o": W_o.astype(NPBF),
            "ga_col": ga_col, "bb_col": ba_col,
            "vscale_col": np.ascontiguousarray(
                ((1.0 - pad[b]) / N).astype(np.float32).reshape(NT, P).T),
            "padout_col": np.ascontiguousarray(
                (1.0 - pad[b]).astype(np.float32).reshape(NT, P).T),
            "posacc": posacc,
        })
    return per_core, (far, kmin_g, kmax_g, k1min, k1max, ts_w)


def kernel(**inputs):
    from concourse.bass_utils import run_bass_kernel_spmd

    per_core, (far, kmin_g, kmax_g, k1min, k1max, ts_w) = _prep_inputs(inputs)
    key = (tuple(far), kmin_g, kmax_g, k1min, k1max, ts_w.tobytes())
    if key not in _cache:
        _cache.clear()
        _cache[key] = _build(ts_w, far, kmin_g, kmax_g, k1min, k1max)
    nc = _cache[key]
    res = run_bass_kernel_spmd(nc, per_core, list(range(B)))
    out = np.stack([res.results[b]["out"] for b in range(B)], axis=0)
    return out.astype(np.float32)


# revision 35
# speedup vs baseline: 1.1653x; 1.0374x over previous
"""HSTU block kernel for Trainium2, 8-core data-parallel over batch.

Key layout/scheduling choices:
  - All PE matmul operands are 16-bit (bf16/f16): 1 cycle/row with fp32 PSUM
    accumulation.  x ships as xT [D, N] bf16 (stats + proj rhs) and row-major
    f32 (+b_o folded in) for the residual.
  - The rel-bias ts_w[bucket(log dt)] reconstruction: y = ln|dt| comes from
    two Act ops (Abs with per-partition bias, then Ln -> f16).  Threshold
    indicator tiles t_k = ck*[y >= th_k] are DVE tensor_scalar ops (4x f16
    mode); their SUM is accumulated on the PE via identity matmuls into PSUM
    together with the pos-bias seed, then copied back over acc.  A slice of
    passes runs on Pool (own accumulator) and a slice accumulates on DVE
    (TensorTensor) -- both folded into the same PSUM chain.
  - Causal masking is baked into the bias (-100 on sub-diagonal cells makes
    silu underflow to 0 in f16) -- no affine_select, no qksil memsets; the
    attn@v matmuls restrict columns to the causal region instead.
  - qk logits are produced transposed (LT [key m, query n]); the rel-bias is
    preloaded into PSUM via an f16 identity matmul so the qk matmul
    accumulates on top of it.  Row tiles r<=3 use 1024-wide PSUM tiles (one
    silu per row tile).
  - PSUM budget (8 banks): stats-stack 2 (four [1,512] accumulators live at
    partition offsets 0/32/64/96 of one bank tile), shared [P,512] pool 2
    (proj/qk/av/repl/out), [P,1024] pool 4 (bias chains + wide qk chunks).
"""

import sys

sys.path.insert(0, "/opt/trn_rl_repo")

import numpy as np
import ml_dtypes

import concourse.bass as bass
import concourse.tile as tile
import concourse.mybir as mybir
from concourse import bacc
from concourse.masks import make_identity

B, N, D = 8, 1024, 512
H, DV, DQ = 8, 64, 64
E = 2 * H * DV + 2 * H * DQ  # 2048
EPS = 1e-5
P = 128
NT = N // P  # 8 row tiles
F32 = mybir.dt.float32
F16 = mybir.dt.float16
BF16 = mybir.dt.bfloat16
NPBF = np.dtype(ml_dtypes.bfloat16)
NEG = -100.0  # baked causal-mask bias: silu(x + NEG) == 0 in f16

# threshold-pass distribution knobs (in units of threshold GROUPS)
PASS_GROUP = 2     # consecutive thresholds merged per pass (bias err <= |ck|)
N_POOL_DIAG = 6    # diag groups on Pool (own chain)

_cache = {}


def _bucket(d):
    d = np.maximum(np.abs(d), 1).astype(np.float32)
    return np.clip((np.log(d) / 0.301).astype(np.int32), 0, 128)


def _plan_chunks(ts, tsq):
    """Uniform-across-batch k-ranges for the threshold passes."""
    far = []  # (r, n0, n1, kmin, kmax)
    for r in range(NT):
        n0 = P * (r + 2)
        while n0 < N:
            n1 = min(((n0 // 512) + 1) * 512, N)
            dmin = int((tsq[:, n0] - ts[:, P * r + P - 1]).min())
            dmax = int((tsq[:, n1 - 1] - ts[:, P * r]).max())
            far.append((r, n0, n1, int(_bucket(dmin)), int(_bucket(dmax))))
            n0 = n1
    # diag band: n in [128r, 128r+128), cells n >= m only
    dmin_g = int((tsq - ts).min())
    dmax_g = 0
    for r in range(NT):
        dmax_g = max(dmax_g, int((tsq[:, P * r + P - 1] - ts[:, P * r]).max()))
    kmin_g, kmax_g = int(_bucket(max(dmin_g, 0))), int(_bucket(dmax_g))
    # band1: n in [128(r+1), 128(r+2)) for r=0..6
    d1min = min(int((tsq[:, P * (r + 1)] - ts[:, P * r + P - 1]).min()) for r in range(NT - 1))
    d1max = max(int((tsq[:, P * (r + 2) - 1] - ts[:, P * r]).max()) for r in range(NT - 1))
    k1min, k1max = int(_bucket(max(d1min, 0))), int(_bucket(d1max))
    return far, kmin_g, kmax_g, k1min, k1max


def _build(ts_w_np, far, kmin_g, kmax_g, k1min, k1max):
    nc = bacc.Bacc()
    d = {}
    for name, shape, dt_ in [
        ("xT", [P, 4 * N], BF16), ("xr", [N, D], F32), ("tsq_rep", [P, N], F32),
        ("uvqk_g", [P, 4 * E], BF16), ("smallpack", [P, 48], F32),
        ("bUv_row", [1, DV * H], BF16), ("W_o", [P, 4 * D], BF16),
        ("posacc", [P, 4608], F16),
    ]:
        d[name] = nc.dram_tensor(name, shape, dt_, kind="ExternalInput")
    out_t = nc.dram_tensor("out", [N, D], F32, kind="ExternalOutput")

    widths = [N - P * r for r in range(NT)]
    offs = np.concatenate([[0], np.cumsum(widths)]).astype(int)
    tsw = ts_w_np.astype(np.float64)
    cks = [float(tsw[k] - tsw[k - 1]) for k in range(1, 129)]
    TH = 0.301  # y = ln|d| threshold scale
    AF = mybir.ActivationFunctionType
    OP = mybir.AluOpType

    def _groups(kmin, kmax, g=PASS_GROUP):
        ks = list(range(kmin + 1, kmax + 1))
        out = []
        i = 0
        while i < len(ks):
            grp = ks[i:i + g]
            out.append((float(TH * grp[0]),
                        float(sum(cks[k - 1] for k in grp))))
            i += g
        return out

    # diag groups: a slice to Pool, the rest to the DVE chain; band1 all DVE;
    # far chunks go through the PE identity-matmul path
    gd_all = _groups(kmin_g, kmax_g)
    n_p = min(N_POOL_DIAG, len(gd_all))
    kp_d = gd_all[len(gd_all) - n_p:]
    kt_d = gd_all[:len(gd_all) - n_p]
    kpe_d = []
    kt_b = _groups(k1min, k1max)
    kpe_b = []

    from contextlib import ExitStack
    with tile.TileContext(nc) as tc, ExitStack() as ctx:
        io = ctx.enter_context(tc.tile_pool(name="io", bufs=1))
        pools = ctx.enter_context(tc.tile_pool(name="work", bufs=4))
        kpool = ctx.enter_context(tc.tile_pool(name="kpool", bufs=8))
        kgpool = ctx.enter_context(tc.tile_pool(name="kgpool", bufs=2))
        pq512 = ctx.enter_context(tc.tile_pool(name="pq512", bufs=2, space="PSUM"))
        pw1024 = ctx.enter_context(tc.tile_pool(name="pw1024", bufs=2, space="PSUM"))
        pstat = ctx.enter_context(tc.tile_pool(name="pstat", bufs=2, space="PSUM"))

        # ---- DMAs (bias-prep inputs first; everything batched) ----
        tsq_rep = io.tile([P, N], F32, tag="tsqr")
        nc.sync.dma_start(tsq_rep[:], d["tsq_rep"][:])
        sp_t = io.tile([P, 48], F32, tag="smallpack")
        nc.sync.dma_start(sp_t[:], d["smallpack"][:])
        bUv_row = io.tile([1, DV * H], BF16, tag="bUv_row")
        nc.sync.dma_start(bUv_row[:], d["bUv_row"][:])
        small = {
            "ntsk_col": sp_t[:, 0:8], "bU_col": sp_t[:, 8:24],
            "ga_col": sp_t[:, 24:28], "bb_col": sp_t[:, 28:32],
            "vscale_col": sp_t[:, 32:40], "padout_col": sp_t[:, 40:48],
            "bUv_row": bUv_row[:],
        }
        xTall = io.tile([P, 4 * N], BF16, tag="xTall")
        nc.sync.dma_start(xTall[:], d["xT"][:])
        xT = [xTall[:, N * s:N * s + N] for s in range(4)]
        uvqkall = io.tile([P, 4 * E], BF16, tag="uvqkall")
        nc.sync.dma_start(uvqkall[:], d["uvqk_g"][:])
        accall = io.tile([P, 4608], F16, tag="accall")
        nc.sync.dma_start(accall[:], d["posacc"][:])
        acc = [accall[:, offs[r]:offs[r + 1]] for r in range(NT)]

        ident = io.tile([P, P], F16, tag="ident")
        make_identity(nc, ident[:])
        ones_col = io.tile([P, 1], BF16, tag="ones_col")
        nc.vector.memset(ones_col[:], 1.0)
        ones_row = io.tile([1, P], BF16, tag="ones_row")
        nc.vector.memset(ones_row[:], 1.0)

        # ---- rel-bias prep: y = ln|tsq - tsk| per row tile, f16 (Act only) ----
        yh = [io.tile([P, widths[r]], F16, tag=f"yh{r}", name=f"yh{r}") for r in range(NT)]
        ystack = io.tile([P, N], F16, tag="ystack")
        ystack2 = io.tile([P, N - P], F16, tag="ystack2")
        for r in range(NT):
            w = widths[r]
            db = pools.tile([P, N], F32, tag="w32", name="db")
            nc.scalar.activation(db[:, :w], tsq_rep[:, P * r:N], AF.Abs,
                                 bias=small["ntsk_col"][:, r:r + 1], scale=1.0)
            nc.scalar.activation(yh[r][:], db[:, :w], AF.Ln)
            nc.vector.tensor_copy(out=ystack[:, P * r:P * r + P], in_=yh[r][:, 0:P])
            if r < NT - 1:
                nc.vector.tensor_copy(out=ystack2[:, P * r:P * r + P], in_=yh[r][:, P:2 * P])

        # ---- layernorm stats of x: four [1,512] accumulators stacked in one
        # PSUM bank at partition offsets 0/32/64/96 ----
        st1 = pstat.tile([P, 512], F32, tag="st", name="st_x")
        st1b = pstat.tile([P, 512], F32, tag="st", name="st_xb")
        srow = [st1[0:1, :], st1[32:33, :], st1[64:65, :], st1b[0:1, :]]
        for s in range(4):
            sq = pools.tile([P, N], BF16, tag="wb16", name="sq")
            nc.vector.tensor_tensor(sq[:], xT[s][:], xT[s][:], OP.mult)
            for c in range(2):
                nc.tensor.matmul(srow[c][:], ones_col[:],
                                 xT[s][:, 512 * c:512 * c + 512],
                                 start=(s == 0), stop=(s == 3))
                nc.tensor.matmul(srow[2 + c][:], ones_col[:],
                                 sq[:, 512 * c:512 * c + 512],
                                 start=(s == 0), stop=(s == 3))
        mu = io.tile([1, N], BF16, tag="mu")
        rs = io.tile([1, N], BF16, tag="rs")
        tmp1 = pools.tile([1, N], BF16, tag="wsm", name="tmp1")
        for c in range(2):
            nc.vector.tensor_scalar_mul(mu[:, 512 * c:512 * c + 512], srow[c][:], 1.0 / D)
            nc.vector.tensor_scalar_mul(tmp1[:, 512 * c:512 * c + 512], srow[2 + c][:], 1.0 / D)
        mu2 = pools.tile([1, N], BF16, tag="wsm", name="mu2")
        nc.vector.tensor_tensor(mu2[:], mu[:], mu[:], OP.mult)
        nc.vector.tensor_tensor(tmp1[:], tmp1[:], mu2[:], OP.subtract)
        nc.vector.tensor_scalar_add(tmp1[:], tmp1[:], EPS)
        nc.scalar.activation(tmp1[:], tmp1[:], AF.Sqrt)
        with nc.allow_low_precision(reason="bf16 rstd is plenty for 2e-2 tol"):
            nc.vector.reciprocal(rs[:], tmp1[:])

        # replicate mu, rs to [P, N] (bf16)
        mur = io.tile([P, N], BF16, tag="mur")
        rsr = io.tile([P, N], BF16, tag="rsr")
        for vec, rep in [(mu, mur), (rs, rsr)]:
            for c in range(2):
                pt = pq512.tile([P, 512], F32, tag="qk", name="rep")
                nc.tensor.matmul(pt[:], ones_row[:], vec[:, 512 * c:512 * c + 512],
                                 start=True, stop=True)
                nc.scalar.copy(out=rep[:, 512 * c:512 * c + 512], in_=pt[:])

        # xn'T = (xT - mu) * rs  (in place, bf16)
        xnt = xT
        for s in range(4):
            nc.vector.tensor_tensor(xnt[s][:], xT[s][:], mur[:], OP.subtract)
            nc.vector.tensor_tensor(xnt[s][:], xnt[s][:], rsr[:], OP.mult)

        # ---- DVE-accumulated threshold chains (emitted early: DVE runs them
        # while PE does the projections) ----
        dacc = io.tile([P, N], F16, tag="dacc")
        for i, (th, cf) in enumerate(kt_d):
            if i == 0:
                nc.vector.tensor_scalar(dacc[:], ystack[:], th, cf,
                                        OP.is_ge, OP.mult)
            else:
                t = kpool.tile([P, N], F16, tag="kt")
                nc.vector.tensor_scalar(t[:], ystack[:], th, cf,
                                        OP.is_ge, OP.mult)
                nc.vector.tensor_tensor(dacc[:], dacc[:], t[:], OP.add)
        dacc2 = io.tile([P, N - P], F16, tag="dacc2")
        for i, (th, cf) in enumerate(kt_b):
            if i == 0:
                nc.vector.tensor_scalar(dacc2[:], ystack2[:], th, cf,
                                        OP.is_ge, OP.mult)
            else:
                t = kpool.tile([P, N], F16, tag="kt")
                nc.vector.tensor_scalar(t[:, :N - P], ystack2[:], th, cf,
                                        OP.is_ge, OP.mult)
                nc.vector.tensor_tensor(dacc2[:], dacc2[:], t[:, :N - P], OP.add)

        # ---- Pool threshold chain (into gacc) ----
        ystack32 = io.tile([P, N], F32, tag="ys32", name="ystack32")
        nc.gpsimd.tensor_copy(out=ystack32[:], in_=ystack[:])
        gacc = io.tile([P, N], F32, tag="gacc", name="gacc")
        for i, (th, cf) in enumerate(kp_d):
            if i == 0:
                nc.gpsimd.tensor_scalar(gacc[:], ystack32[:], th, cf,
                                        OP.is_ge, OP.mult)
            else:
                tg = kgpool.tile([P, N], F32, tag="ktg")
                nc.gpsimd.tensor_scalar(tg[:], ystack32[:], th, cf,
                                        OP.is_ge, OP.mult)
                nc.gpsimd.tensor_tensor(gacc[:], gacc[:], tg[:], OP.add)
        if not kp_d:
            nc.gpsimd.memset(gacc[:], 0.0)

        # ---- projections (PE uninterrupted), then the PE bias chains ----
        projT = {}

        def emit_proj_tile(t_idx, dtype):
            projT[t_idx] = io.tile([P, N], dtype, tag=f"pT{t_idx}", name=f"pT{t_idx}")
            uvs = [uvqkall[:, E * s + P * t_idx:E * s + P * t_idx + P]
                   for s in range(4)]
            for c in range(2):
                pt = pq512.tile([P, 512], F32, tag="qk", name="proj")
                for s in range(4):
                    nc.tensor.matmul(pt[:], uvs[s],
                                     xnt[s][:, 512 * c:512 * c + 512],
                                     start=(s == 0), stop=(s == 3))
                nc.scalar.activation(projT[t_idx][:, 512 * c:512 * c + 512], pt[:],
                                     AF.Silu, bias=small["bU_col"][:, t_idx:t_idx + 1],
                                     scale=1.0)

        for t_idx in range(8, 16):
            emit_proj_tile(t_idx, F16)
        # v row-major, silu + (1-pad)/N scale; bias row folded into the matmul
        vt = [io.tile([P, D], F16, tag=f"v{r}", name=f"v{r}") for r in range(NT)]
        uvv = [uvqkall[:, E * s + 512:E * s + 1024] for s in range(4)]
        for r in range(NT):
            pt = pq512.tile([P, 512], F32, tag="qk", name="projv")
            for s in range(4):
                nc.tensor.matmul(pt[:], xnt[s][:, P * r:P * r + P],
                                 uvv[s], start=(s == 0), stop=False)
            nc.tensor.matmul(pt[:], ones_row[:], small["bUv_row"],
                             start=False, stop=True)
            tmpv = pools.tile([P, D], F16, tag="wv16", name="tmpv")
            nc.scalar.activation(tmpv[:], pt[:], AF.Silu)
            nc.vector.tensor_scalar(vt[r][:], tmpv[:], small["vscale_col"][:, r:r + 1],
                                    None, OP.mult)
        # u projection (consumed only at the final gating multiply)
        for t_idx in range(4):
            emit_proj_tile(t_idx, BF16)

        # diag PE-path: indicator TSPs (DVE) consumed by identity matmuls
        pbd = pw1024.tile([P, N], F32, tag="wide", name="bias_diag")
        diag_started = [False, False]
        for (th, cf) in kpe_d:
            t = kpool.tile([P, N], F16, tag="kt")
            nc.vector.tensor_scalar(t[:], ystack[:], th, cf, OP.is_ge, OP.mult)
            for c in range(2):
                nc.tensor.matmul(pbd[:, 512 * c:512 * c + 512], ident[:],
                                 t[:, 512 * c:512 * c + 512],
                                 start=(not diag_started[c]), stop=False)
                diag_started[c] = True
        # fold gacc (via f16 copy) + dacc + pos seeds into the diag chain
        gacc16 = io.tile([P, N], F16, tag="gacc16")
        nc.scalar.copy(out=gacc16[:], in_=gacc[:])
        for c in range(2):
            if kt_d:
                nc.tensor.matmul(pbd[:, 512 * c:512 * c + 512], ident[:],
                                 dacc[:, 512 * c:512 * c + 512],
                                 start=(not diag_started[c]), stop=False)
                diag_started[c] = True
            if kp_d:
                nc.tensor.matmul(pbd[:, 512 * c:512 * c + 512], ident[:],
                                 gacc16[:, 512 * c:512 * c + 512],
                                 start=(not diag_started[c]), stop=False)
                diag_started[c] = True
        for r in range(NT):
            c = r // 4
            nc.tensor.matmul(pbd[:, P * r:P * r + P], ident[:], acc[r][:, 0:P],
                             start=(not diag_started[c]), stop=(r % 4 == 3))
            diag_started[c] = True
        for r in range(NT):
            nc.scalar.copy(out=acc[r][:, 0:P], in_=pbd[:, P * r:P * r + P])

        # band1 chain
        pbb = pw1024.tile([P, N], F32, tag="wide", name="bias_b1")
        b1_started = [False, False]
        for (th, cf) in kpe_b:
            t = kpool.tile([P, N], F16, tag="kt")
            nc.vector.tensor_scalar(t[:, :N - P], ystack2[:], th, cf,
                                    OP.is_ge, OP.mult)
            for c in range(2):
                w0, w1 = 512 * c, min(512 * c + 512, N - P)
                nc.tensor.matmul(pbb[:, w0:w1], ident[:], t[:, w0:w1],
                                 start=(not b1_started[c]), stop=False)
                b1_started[c] = True
        if kt_b:
            for c in range(2):
                w0, w1 = 512 * c, min(512 * c + 512, N - P)
                nc.tensor.matmul(pbb[:, w0:w1], ident[:], dacc2[:, w0:w1],
                                 start=(not b1_started[c]), stop=False)
                b1_started[c] = True
        for r in range(NT - 1):
            c = r // 4
            nc.tensor.matmul(pbb[:, P * r:P * r + P], ident[:], acc[r][:, P:2 * P],
                             start=(not b1_started[c]), stop=(r % 4 == 3 or r == NT - 2))
            b1_started[c] = True
        for r in range(NT - 1):
            nc.scalar.copy(out=acc[r][:, P:2 * P], in_=pbb[:, P * r:P * r + P])

        # far chunks: per-chunk PSUM accumulation (skip chunks with no passes)
        for (r, n0, n1, kmin, kmax) in far:
            if kmax == kmin:
                continue
            a, b2 = n0 - P * r, n1 - P * r
            w = b2 - a
            pf = pq512.tile([P, 512], F32, tag="qk", name="farc")
            for j, (th, cf) in enumerate(_groups(kmin, kmax)):
                t = kpool.tile([P, N], F16, tag="kt")
                nc.vector.tensor_scalar(t[:, :w], yh[r][:, a:b2], th, cf,
                                        OP.is_ge, OP.mult)
                nc.tensor.matmul(pf[:, :w], ident[:], t[:, :w],
                                 start=(j == 0), stop=False)
            nc.tensor.matmul(pf[:, :w], ident[:], acc[r][:, a:b2],
                             start=False, stop=True)
            nc.scalar.copy(out=acc[r][:, a:b2], in_=pf[:, :w])

        # ---- attention per head ----
        woall = io.tile([P, 4 * D], BF16, tag="woall")
        nc.sync.dma_start(woall[:], d["W_o"][:])
        wo = [woall[:, D * s:D * s + D] for s in range(4)]

        qksil = [io.tile([P, N], F16, tag=f"qs{r}", name=f"qs{r}") for r in range(NT)]
        attnT = [io.tile([P, N], BF16, tag=f"aT{t}", name=f"aT{t}") for t in range(4)]
        st2 = pstat.tile([P, 512], F32, tag="st", name="st_a")
        st2b = pstat.tile([P, 512], F32, tag="st", name="st_ab")
        arow = [st2[0:1, :], st2[32:33, :], st2[64:65, :], st2b[0:1, :]]
        for h in range(H):
            qt = projT[8 + h // 2]
            kt = projT[12 + h // 2]
            pq = 64 * (h % 2)
            for r in range(NT):
                n0 = P * r
                if r < 4:
                    # one wide PSUM tile for the whole row: [n0, 1024)
                    pt = pw1024.tile([P, N], F32, tag="wide", name="qkw")
                    m0 = n0
                    while m0 < N:
                        m1 = min(((m0 // 512) + 1) * 512, N)
                        nc.tensor.matmul(pt[:, m0:m1], ident[:],
                                         acc[r][:, m0 - n0:m1 - n0],
                                         start=True, stop=False)
                        nc.tensor.matmul(pt[:, m0:m1],
                                         kt[pq:pq + 64, P * r:P * r + P],
                                         qt[pq:pq + 64, m0:m1],
                                         start=False, stop=True)
                        m0 = m1
                    nc.scalar.activation(qksil[r][:, n0:N], pt[:, n0:N], AF.Silu)
                else:
                    pt = pq512.tile([P, 512], F32, tag="qk", name="qkn")
                    cw = N - n0
                    nc.tensor.matmul(pt[:, :cw], ident[:], acc[r][:],
                                     start=True, stop=False)
                    nc.tensor.matmul(pt[:, :cw], kt[pq:pq + 64, P * r:P * r + P],
                                     qt[pq:pq + 64, n0:N], start=False, stop=True)
                    nc.scalar.activation(qksil[r][:, n0:N], pt[:, :cw], AF.Silu)
            for c in range(2):
                pa = pq512.tile([P, 512], F32, tag="qk", name="av")
                nsub = min(NT, 4 * (c + 1))
                for r in range(nsub):
                    a = max(0, P * r - 512 * c)
                    nc.tensor.matmul(pa[:64, a:512], vt[r][:, 64 * h:64 * h + 64],
                                     qksil[r][:, 512 * c + a:512 * c + 512],
                                     start=(r == 0), stop=(r == nsub - 1))
                at = attnT[h // 2]
                nc.vector.tensor_copy(out=at[pq:pq + 64, 512 * c:512 * c + 512],
                                      in_=pa[:64, :])
            if h % 2 == 1:
                s = h // 2
                for c in range(2):
                    nc.tensor.matmul(arow[c][:], ones_col[:],
                                     attnT[s][:, 512 * c:512 * c + 512],
                                     start=(s == 0), stop=(s == 3))
                    sqa = pools.tile([P, 512], BF16, tag="wb16", name="sqa")
                    nc.vector.tensor_tensor(sqa[:], attnT[s][:, 512 * c:512 * c + 512],
                                            attnT[s][:, 512 * c:512 * c + 512], OP.mult)
                    nc.tensor.matmul(arow[2 + c][:], ones_col[:], sqa[:],
                                     start=(s == 0), stop=(s == 3))

        # ---- layernorm of attn (stats already accumulated in the heads loop) ----
        mua = io.tile([1, N], BF16, tag="mua")
        rsa = io.tile([1, N], BF16, tag="rsa")
        tmpa = pools.tile([1, N], BF16, tag="wsm", name="tmpa")
        for c in range(2):
            nc.vector.tensor_scalar_mul(mua[:, 512 * c:512 * c + 512], arow[c][:], 1.0 / D)
            nc.vector.tensor_scalar_mul(tmpa[:, 512 * c:512 * c + 512], arow[2 + c][:], 1.0 / D)
        mua2 = pools.tile([1, N], BF16, tag="wsm", name="mua2")
        nc.vector.tensor_tensor(mua2[:], mua[:], mua[:], OP.mult)
        nc.vector.tensor_tensor(tmpa[:], tmpa[:], mua2[:], OP.subtract)
        nc.vector.tensor_scalar_add(tmpa[:], tmpa[:], EPS)
        nc.scalar.activation(tmpa[:], tmpa[:], AF.Sqrt)
        with nc.allow_low_precision(reason="bf16 rstd is plenty for 2e-2 tol"):
            nc.vector.reciprocal(rsa[:], tmpa[:])
        muar = io.tile([P, N], BF16, tag="mur", name="muar")
        rsar = io.tile([P, N], BF16, tag="rsr", name="rsar")
        for vec, rep in [(mua, muar), (rsa, rsar)]:
            for c in range(2):
                pt = pq512.tile([P, 512], F32, tag="qk", name="rep")
                nc.tensor.matmul(pt[:], ones_row[:], vec[:, 512 * c:512 * c + 512],
                                 start=True, stop=True)
                nc.scalar.copy(out=rep[:, 512 * c:512 * c + 512], in_=pt[:])
        # prod = u * (LN_a(attn)*gamma+beta) per column half, then that half's
        # output projection + residual (b_o pre-folded into xr)
        for c in range(2):
            cs = slice(512 * c, 512 * c + 512)
            for s in range(4):
                nc.vector.tensor_tensor(attnT[s][:, cs], attnT[s][:, cs],
                                        muar[:, cs], OP.subtract)
                nc.vector.tensor_tensor(attnT[s][:, cs], attnT[s][:, cs],
                                        rsar[:, cs], OP.mult)
                nc.vector.tensor_scalar(attnT[s][:, cs], attnT[s][:, cs],
                                        small["ga_col"][:, s:s + 1],
                                        small["bb_col"][:, s:s + 1],
                                        OP.mult, OP.add)
                nc.vector.tensor_tensor(attnT[s][:, cs], attnT[s][:, cs],
                                        projT[s][:, cs], OP.mult)
            for t in range(4 * c, 4 * c + 4):
                po = pq512.tile([P, 512], F32, tag="qk", name="outp")
                for s in range(4):
                    nc.tensor.matmul(po[:], attnT[s][:, P * t:P * t + P], wo[s],
                                     start=(s == 0), stop=(s == 3))
                xtile = pools.tile([P, D], F32, tag="w32", name="xtile")
                nc.sync.dma_start(xtile[:], d["xr"][P * t:P * t + P, :])
                ot = pools.tile([P, D], F32, tag="w32", name="ot")
                nc.vector.tensor_tensor(ot[:], po[:], xtile[:], OP.add)
                nc.vector.tensor_scalar(ot[:], ot[:], small["padout_col"][:, t:t + 1],
                                        None, OP.mult)
                nc.sync.dma_start(out_t[P * t:P * t + P, :], ot[:])

    nc.compile()
    return nc


def _prep_inputs(inputs):
    x = np.asarray(inputs["x"], dtype=np.float32)
    ts = np.asarray(inputs["timestamps"]).astype(np.int64)
    pad = np.asarray(inputs["pad_mask"]).astype(np.float32)
    uvqk = np.asarray(inputs["uvqk"], dtype=np.float32)
    W_o = np.asarray(inputs["W_o"], dtype=np.float32)
    b_o = np.asarray(inputs["b_o"], dtype=np.float32)
    gx = np.asarray(inputs["gamma_x"], dtype=np.float32)
    bx = np.asarray(inputs["beta_x"], dtype=np.float32)
    ga = np.asarray(inputs["gamma_a"], dtype=np.float32)
    ba = np.asarray(inputs["beta_a"], dtype=np.float32)
    ts_w = np.asarray(inputs["ts_w"], dtype=np.float32)
    pos_w = np.asarray(inputs["pos_w"], dtype=np.float32)

    tsq = np.concatenate([ts[:, 1:], ts[:, -1:]], axis=1)  # [B, N]
    far, kmin_g, kmax_g, k1min, k1max = _plan_chunks(ts, tsq)

    uvqk_g = (uvqk * gx[:, None]).astype(NPBF)
    bU = bx @ uvqk  # [E]
    bU_col = bU.reshape(E // P, P).T.copy()  # [P, E//P]
    bUv_row = bU[512:1024].reshape(1, 512).astype(NPBF)
    ga_col = ga.reshape(4, P).T.copy()
    ba_col = ba.reshape(4, P).T.copy()

    # pos-bias tiles in [m, n] layout + per-chunk base constants
    widths = [N - P * r for r in range(NT)]
    offs = np.concatenate([[0], np.cumsum(widths)]).astype(int)
    posacc = np.zeros((P, int(offs[-1])), np.float32)
    nidx = np.arange(N)
    pidx = np.arange(P)[:, None]
    for r in range(NT):
        m = P * r + pidx
        nn = nidx[None, P * r:]
        posacc[:, offs[r]:offs[r + 1]] = pos_w[nn - m + (N - 1)]
        posacc[:, offs[r]:offs[r] + P] += ts_w[kmin_g]
        if r < NT - 1:
            posacc[:, offs[r] + P:offs[r] + 2 * P] += ts_w[k1min]
        # causal mask baked in: sub-diagonal cells of the diag block get a
        # large negative bias so silu(qk + bias) underflows to 0 in f16
        sub = pidx > nidx[None, :P]
        posacc[:, offs[r]:offs[r] + P] = np.where(
            sub, NEG, posacc[:, offs[r]:offs[r] + P])
    for (r, n0, n1, kmin, kmax) in far:
        posacc[:, offs[r] + n0 - P * r: offs[r] + n1 - P * r] += ts_w[kmin]
    posacc = posacc.astype(np.float16)

    xr = x + b_o[None, None, :]  # residual rows with b_o folded in

    # packed layouts: one DMA each (row p holds the 4 partition-blocks side
    # by side)
    uvqk_pk = np.ascontiguousarray(
        uvqk_g.reshape(4, P, E).transpose(1, 0, 2).reshape(P, 4 * E))
    wo_pk = np.ascontiguousarray(
        W_o.astype(NPBF).reshape(4, P, D).transpose(1, 0, 2).reshape(P, 4 * D))

    per_core = []
    for b in range(B):
        xT_b = np.ascontiguousarray(x[b].T).astype(NPBF)  # [D, N]
        xT_pk = np.ascontiguousarray(
            xT_b.reshape(4, P, N).transpose(1, 0, 2).reshape(P, 4 * N))
        smallpack = np.concatenate([
            np.ascontiguousarray((-ts[b]).astype(np.float32).reshape(NT, P).T),
            bU_col, ga_col, ba_col,
            np.ascontiguousarray(((1.0 - pad[b]) / N).astype(np.float32).reshape(NT, P).T),
            np.ascontiguousarray((1.0 - pad[b]).astype(np.float32).reshape(NT, P).T),
        ], axis=1).astype(np.float32)
        per_core.append({
            "xT": xT_pk,
            "xr": np.ascontiguousarray(xr[b]),
            "tsq_rep": np.broadcast_to(tsq[b].astype(np.float32), (P, N)).copy(),
            "uvqk_g": uvqk_pk, "smallpack": smallpack, "bUv_row": bUv_row,
            "W_o": wo_pk,
            "posacc": posacc,
        })
    return per_core, (far, kmin_g, kmax_g, k1min, k1max, ts_w)


def kernel(**inputs):
    from concourse.bass_utils import run_bass_kernel_spmd

    per_core, (far, kmin_g, kmax_g, k1min, k1max, ts_w) = _prep_inputs(inputs)
    key = (tuple(far), kmin_g, kmax_g, k1min, k1max, ts_w.tobytes())
    if key not in _cache:
        _cache.clear()
        _cache[key] = _build(ts_w, far, kmin_g, kmax_g, k1min, k1max)
    nc = _cache[key]
    res = run_bass_kernel_spmd(nc, per_core, list(range(B)))
    out = np.stack([res.results[b]["out"] for b in range(B)], axis=0)
    return out.astype(np.float32)


# revision 36
# speedup vs baseline: 1.2263x; 1.0524x over previous
"""HSTU block kernel for Trainium2, 8-core data-parallel over batch.

Key layout/scheduling choices:
  - All PE matmul operands are 16-bit (bf16/f16): 1 cycle/row with fp32 PSUM
    accumulation.  x ships as xT [D, N] bf16 (stats + proj rhs) and row-major
    f32 (+b_o folded in) for the residual.
  - The rel-bias ts_w[bucket(log dt)] reconstruction: y = ln|dt| comes from
    two Act ops (Abs with per-partition bias, then Ln -> f16).  Threshold
    indicator tiles t_k = ck*[y >= th_k] are DVE tensor_scalar ops (4x f16
    mode); their SUM is accumulated on the PE via identity matmuls into PSUM
    together with the pos-bias seed, then copied back over acc.  A slice of
    passes runs on Pool (own accumulator) and a slice accumulates on DVE
    (TensorTensor) -- both folded into the same PSUM chain.
  - Causal masking is baked into the bias (-100 on sub-diagonal cells makes
    silu underflow to 0 in f16) -- no affine_select, no qksil memsets; the
    attn@v matmuls restrict columns to the causal region instead.
  - qk logits are produced transposed (LT [key m, query n]); the rel-bias is
    preloaded into PSUM via an f16 identity matmul so the qk matmul
    accumulates on top of it.  Row tiles r<=3 use 1024-wide PSUM tiles (one
    silu per row tile).
  - PSUM budget (8 banks): stats-stack 2 (four [1,512] accumulators live at
    partition offsets 0/32/64/96 of one bank tile), shared [P,512] pool 2
    (proj/qk/av/repl/out), [P,1024] pool 4 (bias chains + wide qk chunks).
"""

import sys

sys.path.insert(0, "/opt/trn_rl_repo")

import numpy as np
import ml_dtypes

import concourse.bass as bass
import concourse.tile as tile
import concourse.mybir as mybir
from concourse import bacc
from concourse.masks import make_identity

B, N, D = 8, 1024, 512
H, DV, DQ = 8, 64, 64
E = 2 * H * DV + 2 * H * DQ  # 2048
EPS = 1e-5
P = 128
NT = N // P  # 8 row tiles
F32 = mybir.dt.float32
F16 = mybir.dt.float16
BF16 = mybir.dt.bfloat16
NPBF = np.dtype(ml_dtypes.bfloat16)
NEG = -100.0  # baked causal-mask bias: silu(x + NEG) == 0 in f16

# threshold-pass distribution knobs (in units of threshold GROUPS)
PASS_GROUP = 2     # consecutive thresholds merged per pass (bias err <= |ck|)
N_POOL_DIAG = 4    # diag groups on Pool (own chain)

_cache = {}


def _bucket(d):
    d = np.maximum(np.abs(d), 1).astype(np.float32)
    return np.clip((np.log(d) / 0.301).astype(np.int32), 0, 128)


def _plan_chunks(ts, tsq):
    """Uniform-across-batch k-ranges for the threshold passes."""
    far = []  # (r, n0, n1, kmin, kmax)
    for r in range(NT):
        n0 = P * (r + 2)
        while n0 < N:
            n1 = min(((n0 // 512) + 1) * 512, N)
            dmin = int((tsq[:, n0] - ts[:, P * r + P - 1]).min())
            dmax = int((tsq[:, n1 - 1] - ts[:, P * r]).max())
            far.append((r, n0, n1, int(_bucket(dmin)), int(_bucket(dmax))))
            n0 = n1
    # diag band: n in [128r, 128r+128), cells n >= m only
    dmin_g = int((tsq - ts).min())
    dmax_g = 0
    for r in range(NT):
        dmax_g = max(dmax_g, int((tsq[:, P * r + P - 1] - ts[:, P * r]).max()))
    kmin_g, kmax_g = int(_bucket(max(dmin_g, 0))), int(_bucket(dmax_g))
    # band1: n in [128(r+1), 128(r+2)) for r=0..6
    d1min = min(int((tsq[:, P * (r + 1)] - ts[:, P * r + P - 1]).min()) for r in range(NT - 1))
    d1max = max(int((tsq[:, P * (r + 2) - 1] - ts[:, P * r]).max()) for r in range(NT - 1))
    k1min, k1max = int(_bucket(max(d1min, 0))), int(_bucket(d1max))
    return far, kmin_g, kmax_g, k1min, k1max


def _build(ts_w_np, far, kmin_g, kmax_g, k1min, k1max):
    nc = bacc.Bacc()
    d = {}
    for name, shape, dt_ in [
        ("xT", [P, 4 * N], BF16), ("xr", [N, D], F32), ("tsq_rep", [P, N], F32),
        ("uvqk_g", [P, 4 * E], BF16), ("smallpack", [P, 48], F32),
        ("bUv_row", [1, DV * H], BF16), ("W_o", [P, 4 * D], BF16),
        ("posacc", [P, 4608], F16),
    ]:
        d[name] = nc.dram_tensor(name, shape, dt_, kind="ExternalInput")
    out_t = nc.dram_tensor("out", [N, D], F32, kind="ExternalOutput")

    widths = [N - P * r for r in range(NT)]
    offs = np.concatenate([[0], np.cumsum(widths)]).astype(int)
    tsw = ts_w_np.astype(np.float64)
    cks = [float(tsw[k] - tsw[k - 1]) for k in range(1, 129)]
    TH = 0.301  # y = ln|d| threshold scale
    AF = mybir.ActivationFunctionType
    OP = mybir.AluOpType

    def _groups(kmin, kmax, g=PASS_GROUP):
        ks = list(range(kmin + 1, kmax + 1))
        out = []
        i = 0
        while i < len(ks):
            grp = ks[i:i + g]
            out.append((float(TH * grp[0]),
                        float(sum(cks[k - 1] for k in grp))))
            i += g
        return out

    # diag groups: a slice to Pool, the rest to the DVE chain; band1 all DVE;
    # far chunks go through the PE identity-matmul path
    gd_all = _groups(kmin_g, kmax_g)
    n_p = min(N_POOL_DIAG, len(gd_all))
    kp_d = gd_all[len(gd_all) - n_p:]
    kt_d = gd_all[:len(gd_all) - n_p]
    kpe_d = []
    kt_b = _groups(k1min, k1max)
    kpe_b = []

    from contextlib import ExitStack
    with tile.TileContext(nc) as tc, ExitStack() as ctx:
        io = ctx.enter_context(tc.tile_pool(name="io", bufs=1))
        pools = ctx.enter_context(tc.tile_pool(name="work", bufs=4))
        kpool = ctx.enter_context(tc.tile_pool(name="kpool", bufs=8))
        kgpool = ctx.enter_context(tc.tile_pool(name="kgpool", bufs=2))
        pq512 = ctx.enter_context(tc.tile_pool(name="pq512", bufs=2, space="PSUM"))
        pw1024 = ctx.enter_context(tc.tile_pool(name="pw1024", bufs=2, space="PSUM"))
        pstat = ctx.enter_context(tc.tile_pool(name="pstat", bufs=2, space="PSUM"))

        # ---- DMAs (bias-prep inputs first; everything batched) ----
        tsq_rep = io.tile([P, N], F32, tag="tsqr")
        nc.sync.dma_start(tsq_rep[:], d["tsq_rep"][:])
        sp_t = io.tile([P, 48], F32, tag="smallpack")
        nc.sync.dma_start(sp_t[:], d["smallpack"][:])
        bUv_row = io.tile([1, DV * H], BF16, tag="bUv_row")
        nc.sync.dma_start(bUv_row[:], d["bUv_row"][:])
        small = {
            "ntsk_col": sp_t[:, 0:8], "bU_col": sp_t[:, 8:24],
            "ga_col": sp_t[:, 24:28], "bb_col": sp_t[:, 28:32],
            "vscale_col": sp_t[:, 32:40], "padout_col": sp_t[:, 40:48],
            "bUv_row": bUv_row[:],
        }
        xTall = io.tile([P, 4 * N], BF16, tag="xTall")
        nc.sync.dma_start(xTall[:], d["xT"][:])
        xT = [xTall[:, N * s:N * s + N] for s in range(4)]
        uvqkall = io.tile([P, 4 * E], BF16, tag="uvqkall")
        nc.sync.dma_start(uvqkall[:], d["uvqk_g"][:])
        accall = io.tile([P, 4608], F16, tag="accall")
        nc.sync.dma_start(accall[:], d["posacc"][:])
        acc = [accall[:, offs[r]:offs[r + 1]] for r in range(NT)]

        ident = io.tile([P, P], F16, tag="ident")
        make_identity(nc, ident[:])
        ones_col = io.tile([P, 1], BF16, tag="ones_col")
        nc.vector.memset(ones_col[:], 1.0)
        ones_row = io.tile([1, P], BF16, tag="ones_row")
        nc.vector.memset(ones_row[:], 1.0)

        # ---- rel-bias prep: y = ln|tsq - tsk| per row tile, f16 (Act only) ----
        yh = [io.tile([P, widths[r]], F16, tag=f"yh{r}", name=f"yh{r}") for r in range(NT)]
        ystack = io.tile([P, N], F16, tag="ystack")
        ystack2 = io.tile([P, N - P], F16, tag="ystack2")
        for r in range(NT):
            w = widths[r]
            db = pools.tile([P, N], F32, tag="w32", name="db")
            nc.scalar.activation(db[:, :w], tsq_rep[:, P * r:N], AF.Abs,
                                 bias=small["ntsk_col"][:, r:r + 1], scale=1.0)
            nc.scalar.activation(yh[r][:], db[:, :w], AF.Ln)
            nc.vector.tensor_copy(out=ystack[:, P * r:P * r + P], in_=yh[r][:, 0:P])
            if r < NT - 1:
                nc.vector.tensor_copy(out=ystack2[:, P * r:P * r + P], in_=yh[r][:, P:2 * P])

        # ---- layernorm stats of x: four [1,512] accumulators stacked in one
        # PSUM bank at partition offsets 0/32/64/96 ----
        st1 = pstat.tile([P, 512], F32, tag="st", name="st_x")
        st1b = pstat.tile([P, 512], F32, tag="st", name="st_xb")
        srow = [st1[0:1, :], st1[32:33, :], st1[64:65, :], st1b[0:1, :]]
        for s in range(4):
            sq = pools.tile([P, N], BF16, tag="wb16", name="sq")
            nc.vector.tensor_tensor(sq[:], xT[s][:], xT[s][:], OP.mult)
            for c in range(2):
                nc.tensor.matmul(srow[c][:], ones_col[:],
                                 xT[s][:, 512 * c:512 * c + 512],
                                 start=(s == 0), stop=(s == 3))
                nc.tensor.matmul(srow[2 + c][:], ones_col[:],
                                 sq[:, 512 * c:512 * c + 512],
                                 start=(s == 0), stop=(s == 3))
        mu = io.tile([1, N], BF16, tag="mu")
        rs = io.tile([1, N], BF16, tag="rs")
        tmp1 = pools.tile([1, N], BF16, tag="wsm", name="tmp1")
        for c in range(2):
            nc.vector.tensor_scalar_mul(mu[:, 512 * c:512 * c + 512], srow[c][:], 1.0 / D)
            nc.vector.tensor_scalar_mul(tmp1[:, 512 * c:512 * c + 512], srow[2 + c][:], 1.0 / D)
        mu2 = pools.tile([1, N], BF16, tag="wsm", name="mu2")
        nc.vector.tensor_tensor(mu2[:], mu[:], mu[:], OP.mult)
        nc.vector.tensor_tensor(tmp1[:], tmp1[:], mu2[:], OP.subtract)
        nc.vector.tensor_scalar_add(tmp1[:], tmp1[:], EPS)
        nc.scalar.activation(tmp1[:], tmp1[:], AF.Sqrt)
        with nc.allow_low_precision(reason="bf16 rstd is plenty for 2e-2 tol"):
            nc.vector.reciprocal(rs[:], tmp1[:])

        # replicate mu, rs to [P, N] (bf16)
        mur = io.tile([P, N], BF16, tag="mur")
        rsr = io.tile([P, N], BF16, tag="rsr")
        for vec, rep in [(mu, mur), (rs, rsr)]:
            for c in range(2):
                pt = pq512.tile([P, 512], F32, tag="qk", name="rep")
                nc.tensor.matmul(pt[:], ones_row[:], vec[:, 512 * c:512 * c + 512],
                                 start=True, stop=True)
                nc.scalar.copy(out=rep[:, 512 * c:512 * c + 512], in_=pt[:])

        # xn'T = (xT - mu) * rs  (in place, bf16)
        xnt = xT
        for s in range(4):
            nc.vector.tensor_tensor(xnt[s][:], xT[s][:], mur[:], OP.subtract)
            nc.vector.tensor_tensor(xnt[s][:], xnt[s][:], rsr[:], OP.mult)

        # ---- DVE-accumulated threshold chains (emitted early: DVE runs them
        # while PE does the projections) ----
        dacc = io.tile([P, N], F16, tag="dacc")
        for i, (th, cf) in enumerate(kt_d):
            if i == 0:
                nc.vector.tensor_scalar(dacc[:], ystack[:], th, cf,
                                        OP.is_ge, OP.mult)
            else:
                t = kpool.tile([P, N], F16, tag="kt")
                nc.vector.tensor_scalar(t[:], ystack[:], th, cf,
                                        OP.is_ge, OP.mult)
                nc.vector.tensor_tensor(dacc[:], dacc[:], t[:], OP.add)
        dacc2 = io.tile([P, N - P], F16, tag="dacc2")
        for i, (th, cf) in enumerate(kt_b):
            if i == 0:
                nc.vector.tensor_scalar(dacc2[:], ystack2[:], th, cf,
                                        OP.is_ge, OP.mult)
            else:
                t = kpool.tile([P, N], F16, tag="kt")
                nc.vector.tensor_scalar(t[:, :N - P], ystack2[:], th, cf,
                                        OP.is_ge, OP.mult)
                nc.vector.tensor_tensor(dacc2[:], dacc2[:], t[:, :N - P], OP.add)

        # ---- Pool threshold chain (into gacc) ----
        ystack32 = io.tile([P, N], F32, tag="ys32", name="ystack32")
        nc.gpsimd.tensor_copy(out=ystack32[:], in_=ystack[:])
        gacc = io.tile([P, N], F32, tag="gacc", name="gacc")
        for i, (th, cf) in enumerate(kp_d):
            if i == 0:
                nc.gpsimd.tensor_scalar(gacc[:], ystack32[:], th, cf,
                                        OP.is_ge, OP.mult)
            else:
                tg = kgpool.tile([P, N], F32, tag="ktg")
                nc.gpsimd.tensor_scalar(tg[:], ystack32[:], th, cf,
                                        OP.is_ge, OP.mult)
                nc.gpsimd.tensor_tensor(gacc[:], gacc[:], tg[:], OP.add)
        if not kp_d:
            nc.gpsimd.memset(gacc[:], 0.0)

        # ---- projections (PE uninterrupted), then the PE bias chains ----
        projT = {}

        def emit_proj_tile(t_idx, dtype):
            projT[t_idx] = io.tile([P, N], dtype, tag=f"pT{t_idx}", name=f"pT{t_idx}")
            uvs = [uvqkall[:, E * s + P * t_idx:E * s + P * t_idx + P]
                   for s in range(4)]
            for c in range(2):
                pt = pq512.tile([P, 512], F32, tag="qk", name="proj")
                for s in range(4):
                    nc.tensor.matmul(pt[:], uvs[s],
                                     xnt[s][:, 512 * c:512 * c + 512],
                                     start=(s == 0), stop=(s == 3))
                nc.scalar.activation(projT[t_idx][:, 512 * c:512 * c + 512], pt[:],
                                     AF.Silu, bias=small["bU_col"][:, t_idx:t_idx + 1],
                                     scale=1.0)

        for t_idx in range(8, 16):
            emit_proj_tile(t_idx, F16)
        # v row-major, silu + (1-pad)/N scale; bias row folded into the matmul
        vt = [io.tile([P, D], F16, tag=f"v{r}", name=f"v{r}") for r in range(NT)]
        uvv = [uvqkall[:, E * s + 512:E * s + 1024] for s in range(4)]
        for r in range(NT):
            pt = pq512.tile([P, 512], F32, tag="qk", name="projv")
            for s in range(4):
                nc.tensor.matmul(pt[:], xnt[s][:, P * r:P * r + P],
                                 uvv[s], start=(s == 0), stop=False)
            nc.tensor.matmul(pt[:], ones_row[:], small["bUv_row"],
                             start=False, stop=True)
            tmpv = pools.tile([P, D], F16, tag="wv16", name="tmpv")
            nc.scalar.activation(tmpv[:], pt[:], AF.Silu)
            nc.vector.tensor_scalar(vt[r][:], tmpv[:], small["vscale_col"][:, r:r + 1],
                                    None, OP.mult)
        # u projection (consumed only at the final gating multiply)
        for t_idx in range(4):
            emit_proj_tile(t_idx, BF16)

        # diag PE-path: indicator TSPs (DVE) consumed by identity matmuls
        pbd = pw1024.tile([P, N], F32, tag="wide", name="bias_diag")
        diag_started = [False, False]
        for (th, cf) in kpe_d:
            t = kpool.tile([P, N], F16, tag="kt")
            nc.vector.tensor_scalar(t[:], ystack[:], th, cf, OP.is_ge, OP.mult)
            for c in range(2):
                nc.tensor.matmul(pbd[:, 512 * c:512 * c + 512], ident[:],
                                 t[:, 512 * c:512 * c + 512],
                                 start=(not diag_started[c]), stop=False)
                diag_started[c] = True
        # fold gacc (via f16 copy) + dacc + pos seeds into the diag chain
        gacc16 = io.tile([P, N], F16, tag="gacc16")
        nc.vector.tensor_copy(out=gacc16[:], in_=gacc[:])
        for c in range(2):
            if kt_d:
                nc.tensor.matmul(pbd[:, 512 * c:512 * c + 512], ident[:],
                                 dacc[:, 512 * c:512 * c + 512],
                                 start=(not diag_started[c]), stop=False)
                diag_started[c] = True
            if kp_d:
                nc.tensor.matmul(pbd[:, 512 * c:512 * c + 512], ident[:],
                                 gacc16[:, 512 * c:512 * c + 512],
                                 start=(not diag_started[c]), stop=False)
                diag_started[c] = True
        for r in range(NT):
            c = r // 4
            nc.tensor.matmul(pbd[:, P * r:P * r + P], ident[:], acc[r][:, 0:P],
                             start=(not diag_started[c]), stop=(r % 4 == 3))
            diag_started[c] = True
        for r in range(NT):
            nc.scalar.copy(out=acc[r][:, 0:P], in_=pbd[:, P * r:P * r + P])

        # band1 chain
        pbb = pw1024.tile([P, N], F32, tag="wide", name="bias_b1")
        b1_started = [False, False]
        for (th, cf) in kpe_b:
            t = kpool.tile([P, N], F16, tag="kt")
            nc.vector.tensor_scalar(t[:, :N - P], ystack2[:], th, cf,
                                    OP.is_ge, OP.mult)
            for c in range(2):
                w0, w1 = 512 * c, min(512 * c + 512, N - P)
                nc.tensor.matmul(pbb[:, w0:w1], ident[:], t[:, w0:w1],
                                 start=(not b1_started[c]), stop=False)
                b1_started[c] = True
        if kt_b:
            for c in range(2):
                w0, w1 = 512 * c, min(512 * c + 512, N - P)
                nc.tensor.matmul(pbb[:, w0:w1], ident[:], dacc2[:, w0:w1],
                                 start=(not b1_started[c]), stop=False)
                b1_started[c] = True
        for r in range(NT - 1):
            c = r // 4
            nc.tensor.matmul(pbb[:, P * r:P * r + P], ident[:], acc[r][:, P:2 * P],
                             start=(not b1_started[c]), stop=(r % 4 == 3 or r == NT - 2))
            b1_started[c] = True
        for r in range(NT - 1):
            nc.scalar.copy(out=acc[r][:, P:2 * P], in_=pbb[:, P * r:P * r + P])

        # far chunks: per-chunk PSUM accumulation (skip chunks with no passes)
        for (r, n0, n1, kmin, kmax) in far:
            if kmax == kmin:
                continue
            a, b2 = n0 - P * r, n1 - P * r
            w = b2 - a
            pf = pq512.tile([P, 512], F32, tag="qk", name="farc")
            for j, (th, cf) in enumerate(_groups(kmin, kmax)):
                t = kpool.tile([P, N], F16, tag="kt")
                nc.vector.tensor_scalar(t[:, :w], yh[r][:, a:b2], th, cf,
                                        OP.is_ge, OP.mult)
                nc.tensor.matmul(pf[:, :w], ident[:], t[:, :w],
                                 start=(j == 0), stop=False)
            nc.tensor.matmul(pf[:, :w], ident[:], acc[r][:, a:b2],
                             start=False, stop=True)
            nc.scalar.copy(out=acc[r][:, a:b2], in_=pf[:, :w])

        # ---- attention per head ----
        woall = io.tile([P, 4 * D], BF16, tag="woall")
        nc.sync.dma_start(woall[:], d["W_o"][:])
        wo = [woall[:, D * s:D * s + D] for s in range(4)]

        qksil = [io.tile([P, N], F16, tag=f"qs{r}", name=f"qs{r}") for r in range(NT)]
        attnT = [io.tile([P, N], BF16, tag=f"aT{t}", name=f"aT{t}") for t in range(4)]
        st2 = pstat.tile([P, 512], F32, tag="st", name="st_a")
        st2b = pstat.tile([P, 512], F32, tag="st", name="st_ab")
        arow = [st2[0:1, :], st2[32:33, :], st2[64:65, :], st2b[0:1, :]]
        for h in range(H):
            qt = projT[8 + h // 2]
            kt = projT[12 + h // 2]
            pq = 64 * (h % 2)
            for r in range(NT):
                n0 = P * r
                if r < 4:
                    # one wide PSUM tile for the whole row: [n0, 1024)
                    pt = pw1024.tile([P, N], F32, tag="wide", name="qkw")
                    m0 = n0
                    while m0 < N:
                        m1 = min(((m0 // 512) + 1) * 512, N)
                        nc.tensor.matmul(pt[:, m0:m1], ident[:],
                                         acc[r][:, m0 - n0:m1 - n0],
                                         start=True, stop=False)
                        nc.tensor.matmul(pt[:, m0:m1],
                                         kt[pq:pq + 64, P * r:P * r + P],
                                         qt[pq:pq + 64, m0:m1],
                                         start=False, stop=True)
                        m0 = m1
                    nc.scalar.activation(qksil[r][:, n0:N], pt[:, n0:N], AF.Silu)
                else:
                    pt = pq512.tile([P, 512], F32, tag="qk", name="qkn")
                    cw = N - n0
                    nc.tensor.matmul(pt[:, :cw], ident[:], acc[r][:],
                                     start=True, stop=False)
                    nc.tensor.matmul(pt[:, :cw], kt[pq:pq + 64, P * r:P * r + P],
                                     qt[pq:pq + 64, n0:N], start=False, stop=True)
                    nc.scalar.activation(qksil[r][:, n0:N], pt[:, :cw], AF.Silu)
            for c in range(2):
                pa = pq512.tile([P, 512], F32, tag="qk", name="av")
                nsub = min(NT, 4 * (c + 1))
                for r in range(nsub):
                    a = max(0, P * r - 512 * c)
                    nc.tensor.matmul(pa[:64, a:512], vt[r][:, 64 * h:64 * h + 64],
                                     qksil[r][:, 512 * c + a:512 * c + 512],
                                     start=(r == 0), stop=(r == nsub - 1))
                at = attnT[h // 2]
                nc.vector.tensor_copy(out=at[pq:pq + 64, 512 * c:512 * c + 512],
                                      in_=pa[:64, :])
            if h % 2 == 1:
                s = h // 2
                for c in range(2):
                    nc.tensor.matmul(arow[c][:], ones_col[:],
                                     attnT[s][:, 512 * c:512 * c + 512],
                                     start=(s == 0), stop=(s == 3))
                    sqa = pools.tile([P, 512], BF16, tag="wb16", name="sqa")
                    nc.vector.tensor_tensor(sqa[:], attnT[s][:, 512 * c:512 * c + 512],
                                            attnT[s][:, 512 * c:512 * c + 512], OP.mult)
                    nc.tensor.matmul(arow[2 + c][:], ones_col[:], sqa[:],
                                     start=(s == 0), stop=(s == 3))

        # ---- layernorm of attn (stats already accumulated in the heads loop) ----
        mua = io.tile([1, N], BF16, tag="mua")
        rsa = io.tile([1, N], BF16, tag="rsa")
        tmpa = pools.tile([1, N], BF16, tag="wsm", name="tmpa")
        for c in range(2):
            nc.vector.tensor_scalar_mul(mua[:, 512 * c:512 * c + 512], arow[c][:], 1.0 / D)
            nc.vector.tensor_scalar_mul(tmpa[:, 512 * c:512 * c + 512], arow[2 + c][:], 1.0 / D)
        mua2 = pools.tile([1, N], BF16, tag="wsm", name="mua2")
        nc.vector.tensor_tensor(mua2[:], mua[:], mua[:], OP.mult)
        nc.vector.tensor_tensor(tmpa[:], tmpa[:], mua2[:], OP.subtract)
        nc.vector.tensor_scalar_add(tmpa[:], tmpa[:], EPS)
        nc.scalar.activation(tmpa[:], tmpa[:], AF.Sqrt)
        with nc.allow_low_precision(reason="bf16 rstd is plenty for 2e-2 tol"):
            nc.vector.reciprocal(rsa[:], tmpa[:])
        muar = io.tile([P, N], BF16, tag="mur", name="muar")
        rsar = io.tile([P, N], BF16, tag="rsr", name="rsar")
        for vec, rep in [(mua, muar), (rsa, rsar)]:
            for c in range(2):
                pt = pq512.tile([P, 512], F32, tag="qk", name="rep")
                nc.tensor.matmul(pt[:], ones_row[:], vec[:, 512 * c:512 * c + 512],
                                 start=True, stop=True)
                nc.scalar.copy(out=rep[:, 512 * c:512 * c + 512], in_=pt[:])
        # prod = u * (LN_a(attn)*gamma+beta) per column half, then that half's
        # output projection + residual (b_o pre-folded into xr)
        for c in range(2):
            cs = slice(512 * c, 512 * c + 512)
            for s in range(4):
                nc.vector.tensor_tensor(attnT[s][:, cs], attnT[s][:, cs],
                                        muar[:, cs], OP.subtract)
                nc.vector.tensor_tensor(attnT[s][:, cs], attnT[s][:, cs],
                                        rsar[:, cs], OP.mult)
                nc.vector.tensor_scalar(attnT[s][:, cs], attnT[s][:, cs],
                                        small["ga_col"][:, s:s + 1],
                                        small["bb_col"][:, s:s + 1],
                                        OP.mult, OP.add)
                nc.vector.tensor_tensor(attnT[s][:, cs], attnT[s][:, cs],
                                        projT[s][:, cs], OP.mult)
            for t in range(4 * c, 4 * c + 4):
                po = pq512.tile([P, 512], F32, tag="qk", name="outp")
                for s in range(4):
                    nc.tensor.matmul(po[:], attnT[s][:, P * t:P * t + P], wo[s],
                                     start=(s == 0), stop=(s == 3))
                xtile = pools.tile([P, D], F32, tag="w32", name="xtile")
                nc.sync.dma_start(xtile[:], d["xr"][P * t:P * t + P, :])
                ot = pools.tile([P, D], F32, tag="w32", name="ot")
                nc.vector.tensor_tensor(ot[:], po[:], xtile[:], OP.add)
                nc.vector.tensor_scalar(ot[:], ot[:], small["padout_col"][:, t:t + 1],
                                        None, OP.mult)
                nc.sync.dma_start(out_t[P * t:P * t + P, :], ot[:])

    nc.compile()
    return nc


def _prep_inputs(inputs):
    x = np.asarray(inputs["x"], dtype=np.float32)
    ts = np.asarray(inputs["timestamps"]).astype(np.int64)
    pad = np.asarray(inputs["pad_mask"]).astype(np.float32)
    uvqk = np.asarray(inputs["uvqk"], dtype=np.float32)
    W_o = np.asarray(inputs["W_o"], dtype=np.float32)
    b_o = np.asarray(inputs["b_o"], dtype=np.float32)
    gx = np.asarray(inputs["gamma_x"], dtype=np.float32)
    bx = np.asarray(inputs["beta_x"], dtype=np.float32)
    ga = np.asarray(inputs["gamma_a"], dtype=np.float32)
    ba = np.asarray(inputs["beta_a"], dtype=np.float32)
    ts_w = np.asarray(inputs["ts_w"], dtype=np.float32)
    pos_w = np.asarray(inputs["pos_w"], dtype=np.float32)

    tsq = np.concatenate([ts[:, 1:], ts[:, -1:]], axis=1)  # [B, N]
    far, kmin_g, kmax_g, k1min, k1max = _plan_chunks(ts, tsq)

    uvqk_g = (uvqk * gx[:, None]).astype(NPBF)
    bU = bx @ uvqk  # [E]
    bU_col = bU.reshape(E // P, P).T.copy()  # [P, E//P]
    bUv_row = bU[512:1024].reshape(1, 512).astype(NPBF)
    ga_col = ga.reshape(4, P).T.copy()
    ba_col = ba.reshape(4, P).T.copy()

    # pos-bias tiles in [m, n] layout + per-chunk base constants
    widths = [N - P * r for r in range(NT)]
    offs = np.concatenate([[0], np.cumsum(widths)]).astype(int)
    posacc = np.zeros((P, int(offs[-1])), np.float32)
    nidx = np.arange(N)
    pidx = np.arange(P)[:, None]
    for r in range(NT):
        m = P * r + pidx
        nn = nidx[None, P * r:]
        posacc[:, offs[r]:offs[r + 1]] = pos_w[nn - m + (N - 1)]
        posacc[:, offs[r]:offs[r] + P] += ts_w[kmin_g]
        if r < NT - 1:
            posacc[:, offs[r] + P:offs[r] + 2 * P] += ts_w[k1min]
        # causal mask baked in: sub-diagonal cells of the diag block get a
        # large negative bias so silu(qk + bias) underflows to 0 in f16
        sub = pidx > nidx[None, :P]
        posacc[:, offs[r]:offs[r] + P] = np.where(
            sub, NEG, posacc[:, offs[r]:offs[r] + P])
    for (r, n0, n1, kmin, kmax) in far:
        posacc[:, offs[r] + n0 - P * r: offs[r] + n1 - P * r] += ts_w[kmin]
    posacc = posacc.astype(np.float16)

    xr = x + b_o[None, None, :]  # residual rows with b_o folded in

    # packed layouts: one DMA each (row p holds the 4 partition-blocks side
    # by side)
    uvqk_pk = np.ascontiguousarray(
        uvqk_g.reshape(4, P, E).transpose(1, 0, 2).reshape(P, 4 * E))
    wo_pk = np.ascontiguousarray(
        W_o.astype(NPBF).reshape(4, P, D).transpose(1, 0, 2).reshape(P, 4 * D))

    per_core = []
    for b in range(B):
        xT_b = np.ascontiguousarray(x[b].T).astype(NPBF)  # [D, N]
        xT_pk = np.ascontiguousarray(
            xT_b.reshape(4, P, N).transpose(1, 0, 2).reshape(P, 4 * N))
        smallpack = np.concatenate([
            np.ascontiguousarray((-ts[b]).astype(np.float32).reshape(NT, P).T),
            bU_col, ga_col, ba_col,
            np.ascontiguousarray(((1.0 - pad[b]) / N).astype(np.float32).reshape(NT, P).T),
            np.ascontiguousarray((1.0 - pad[b]).astype(np.float32).reshape(NT, P).T),
        ], axis=1).astype(np.float32)
        per_core.append({
            "xT": xT_pk,
            "xr": np.ascontiguousarray(xr[b]),
            "tsq_rep": np.broadcast_to(tsq[b].astype(np.float32), (P, N)).copy(),
            "uvqk_g": uvqk_pk, "smallpack": smallpack, "bUv_row": bUv_row,
            "W_o": wo_pk,
            "posacc": posacc,
        })
    return per_core, (far, kmin_g, kmax_g, k1min, k1max, ts_w)


def kernel(**inputs):
    from concourse.bass_utils import run_bass_kernel_spmd

    per_core, (far, kmin_g, kmax_g, k1min, k1max, ts_w) = _prep_inputs(inputs)
    key = (tuple(far), kmin_g, kmax_g, k1min, k1max, ts_w.tobytes())
    if key not in _cache:
        _cache.clear()
        _cache[key] = _build(ts_w, far, kmin_g, kmax_g, k1min, k1max)
    nc = _cache[key]
    res = run_bass_kernel_spmd(nc, per_core, list(range(B)))
    out = np.stack([res.results[b]["out"] for b in range(B)], axis=0)
    return out.astype(np.float32)
